# revision 1
# baseline (speedup 1.0000x reference)
"""Trainium2 Bass kernel for the DigitalTwinModel (3-layer LSTM digital twin).

Strategy: 8-way model parallelism over the hidden dimension.
  - All activations live feature-major on chip: [features(part), batch(free)],
    batch N=256 so fp32r matmuls run at full (1 cycle/row) speed.
  - Core k owns hidden-feature slice k*128:(k+1)*128 of every LSTM layer's
    h/c state, and the matching 4x128 rows of W_ih / W_hh (gate order
    i,f,g,o). Weights stay resident in SBUF for the whole kernel
    (~20 MB/core) - nothing is re-streamed from HBM inside the loop.
  - After each layer's elementwise, the 128-row h' slice is AllGathered
    ([128,256] per rank -> [1024,256]) so the next matmul can contract over
    the full hidden dim. 3 AllGathers per timestep; the W_hh @ h_l(t)
    matmuls for step t+1 are issued inside the AllGather windows so the
    TensorEngine stays busy while ncfw moves bytes:
      AG0(t) window: batch-major output write of step t-1
      AG1(t) window: W_hh[0] @ h0(t)
      AG2(t) window: W_hh[1] @ h1(t)
      decoder tail:  W_hh[2] @ h2(t)
  - Decoder (Wd1 relu, Wd2, re-encode We relu) is replicated on every core
    (cheaper than more collectives). The model output is additionally
    computed batch-major via matmul(lhsT=rT, rhs=Wd2T) = r @ Wd2.T so it
    can be DMA'd straight into out[:, t, :] with no transpose; bd2 is
    added on the host.
"""

import numpy as np

import concourse.bass as bass
import concourse.mybir as mybir
from concourse import bacc
import concourse.tile as tile
from concourse.bass_utils import run_bass_kernel_spmd

F32 = mybir.dt.float32
F32R = mybir.dt.float32r
AF = mybir.ActivationFunctionType

B, D_IN, H, L, T = 256, 512, 1024, 3, 32
NCORES = 8
P = 128
SH = H // NCORES          # 128 hidden features owned per core
KT_H = H // P             # 8 k-tiles over hidden dim
KT_D = D_IN // P          # 4 k-tiles over model-output dim
MT_G = 4 * SH // P        # 4 m-tiles of gates per core (one per gate)
GROUPS = [list(range(NCORES))]


def _r(ap):
    """Bitcast an fp32 AP to float32r for full-rate PE matmuls."""
    return ap.bitcast(F32R)


def _pe_touch(nc, ap2d):
    """Tiny ldweights that makes the PE observe a tile's producer semaphore.

    Fused fp32r matmuls have a single sync-wait slot in the ISA; when a
    matmul would need two waits (e.g. fresh-DMA rhs + a PSUM WAR), walrus
    codegen fails. A 1x4 ldweights touch reads the tile on the PE stream
    and absorbs the wait; the junk weights are replaced by the next fused
    matmul's internal weight load.
    """
    nc.tensor.ldweights(weights=ap2d[0:1, 0:2].bitcast(mybir.dt.bfloat16))


def build_program(timesteps=T):
    # default 16 KiB/partition dynamic-DMA scratch is dead weight here (no
    # indirect DMA in this kernel) - reclaim most of it for tiles.
    nc = bacc.Bacc(None, num_devices=NCORES, dynamic_dma_scratch_size=2048)

    # ---- kernel I/O (per-core payloads supplied from the host) ----
    wih = [nc.dram_tensor(f"wih{l}", [H, 4 * SH], F32R, kind="ExternalInput") for l in range(L)]
    whh = [nc.dram_tensor(f"whh{l}", [H, 4 * SH], F32R, kind="ExternalInput") for l in range(L)]
    bg = [nc.dram_tensor(f"bg{l}", [P, MT_G], F32, kind="ExternalInput") for l in range(L)]
    wd1 = nc.dram_tensor("wd1", [H, H], F32R, kind="ExternalInput")
    wd2 = nc.dram_tensor("wd2", [H, D_IN], F32R, kind="ExternalInput")
    we = nc.dram_tensor("we", [D_IN, H], F32R, kind="ExternalInput")
    bd1 = nc.dram_tensor("bd1", [P, KT_H], F32, kind="ExternalInput")
    bd2 = nc.dram_tensor("bd2", [P, KT_D], F32, kind="ExternalInput")
    be = nc.dram_tensor("be", [P, KT_H], F32, kind="ExternalInput")
    enc0 = nc.dram_tensor("enc0", [H, B], F32R, kind="ExternalInput")
    out = nc.dram_tensor("out", [B, timesteps, D_IN], F32, kind="ExternalOutput")

    with tile.TileContext(nc) as tc:
        with (
            tc.tile_pool(name="singles", bufs=1) as singles,
            tc.tile_pool(name="acts", bufs=1) as acts,
            tc.tile_pool(name="gtmp", bufs=1) as gtmp,
            tc.tile_pool(name="hloc", bufs=2) as hlocp,
            tc.tile_pool(name="obuf", bufs=1) as obuf,
            tc.tile_pool(name="pg", bufs=1, space="PSUM") as pgp,
            tc.tile_pool(name="pwork", bufs=2, space="PSUM") as pwork,
            tc.tile_pool(name="dram", bufs=2, space="DRAM") as dram,
        ):
            # ---- load resident weights/biases into SBUF ----
            s_wih, s_whh, s_bg = [], [], []
            for l in range(L):
                w = singles.tile([P, KT_H, 4 * SH], F32R, tag=f"swih{l}", name=f"swih{l}")
                nc.sync.dma_start(out=w, in_=wih[l][:].rearrange("(kk p) m -> p kk m", p=P))
                _pe_touch(nc, w[:, 0, :])
                s_wih.append(w)
            for l in range(L):
                w = singles.tile([P, KT_H, 4 * SH], F32R, tag=f"swhh{l}", name=f"swhh{l}")
                nc.sync.dma_start(out=w, in_=whh[l][:].rearrange("(kk p) m -> p kk m", p=P))
                _pe_touch(nc, w[:, 0, :])
                s_whh.append(w)
            for l in range(L):
                t_ = singles.tile([P, MT_G], F32, tag=f"sbg{l}", name=f"sbg{l}")
                nc.sync.dma_start(out=t_, in_=bg[l][:])
                s_bg.append(t_)
            s_wd1 = singles.tile([P, KT_H, H], F32R, tag="swd1", name="swd1")
            nc.sync.dma_start(out=s_wd1, in_=wd1[:].rearrange("(kk p) m -> p kk m", p=P))
            _pe_touch(nc, s_wd1[:, 0, :])
            s_wd2 = singles.tile([P, KT_H, D_IN], F32R, tag="swd2", name="swd2")
            nc.sync.dma_start(out=s_wd2, in_=wd2[:].rearrange("(kk p) m -> p kk m", p=P))
            _pe_touch(nc, s_wd2[:, 0, :])
            s_we = singles.tile([P, KT_D, H], F32R, tag="swe", name="swe")
            nc.sync.dma_start(out=s_we, in_=we[:].rearrange("(kk p) m -> p kk m", p=P))
            _pe_touch(nc, s_we[:, 0, :])
            s_bd1 = singles.tile([P, KT_H], F32, tag="sbd1", name="sbd1")
            nc.sync.dma_start(out=s_bd1, in_=bd1[:])
            s_bd2 = singles.tile([P, KT_D], F32, tag="sbd2", name="sbd2")
            nc.sync.dma_start(out=s_bd2, in_=bd2[:])
            s_be = singles.tile([P, KT_H], F32, tag="sbe", name="sbe")
            nc.sync.dma_start(out=s_be, in_=be[:])

            # persistent cell state (zero-initialised)
            s_c = []
            for l in range(L):
                c = singles.tile([P, B], F32, tag=f"c{l}", name=f"c{l}")
                nc.vector.memset(c, 0.0)
                s_c.append(c)

            def gate_mms(pg_t, w, rhs, first, last):
                """Accumulate w^T @ rhs into the 4 gate m-tiles of pg_t.

                PSUM `start=True` clears has_written for the whole 2 KiB bank
                (zero region), and two gate m-tiles share each bank - so only
                the bank-FIRST gate (m even) opens the group and only the
                bank-LAST gate (m odd) closes it. The bank-wide clear from the
                even gate's start covers the odd gate's region, whose first
                write then lands in overwrite mode per the has_written bits.
                """
                for kk in range(KT_H):
                    for m in range(MT_G):
                        nc.tensor.matmul(
                            pg_t[:, m, :],
                            lhsT=(w[:, kk, m * P:(m + 1) * P]),
                            rhs=(rhs[:, kk, :]),
                            start=(first and kk == 0 and m % 2 == 0),
                            stop=(last and kk == KT_H - 1 and m % 2 == 1),
                        )

            def elementwise(l, pg_t, first_step):
                """gates -> (h'_k slice, updated c)."""
                ti = gtmp.tile([P, B], F32, tag="ti", name="ti")
                tg = gtmp.tile([P, B], F32, tag="tg", name="tg")
                to = gtmp.tile([P, B], F32, tag="to", name="to")
                nc.scalar.activation(ti, pg_t[:, 0, :], AF.Sigmoid, bias=s_bg[l][:, 0:1])
                if first_step:
                    nc.scalar.activation(tg, pg_t[:, 2, :], AF.Tanh, bias=s_bg[l][:, 2:3])
                    nc.scalar.activation(to, pg_t[:, 3, :], AF.Sigmoid, bias=s_bg[l][:, 3:4])
                    # c = 0 -> c_new = i*g
                    nc.vector.tensor_mul(s_c[l], ti, tg)
                else:
                    tf = gtmp.tile([P, B], F32, tag="tf", name="tf")
                    t1 = gtmp.tile([P, B], F32, tag="t1", name="t1")
                    t2 = gtmp.tile([P, B], F32, tag="t2", name="t2")
                    nc.scalar.activation(tg, pg_t[:, 2, :], AF.Tanh, bias=s_bg[l][:, 2:3])
                    nc.vector.tensor_mul(t1, ti, tg)       # i * g
                    nc.scalar.activation(tf, pg_t[:, 1, :], AF.Sigmoid, bias=s_bg[l][:, 1:2])
                    nc.vector.tensor_mul(t2, tf, s_c[l])   # f * c
                    nc.scalar.activation(to, pg_t[:, 3, :], AF.Sigmoid, bias=s_bg[l][:, 3:4])
                    nc.vector.tensor_add(s_c[l], t1, t2)
                # tanh(c) -> reuse tg slot (dead after i*g)
                tanhc = gtmp.tile([P, B], F32, tag="tg", name="tg")
                nc.scalar.activation(tanhc, s_c[l], AF.Tanh)
                hl = hlocp.tile([P, B], F32R, tag="hl", name="hl")
                nc.vector.tensor_mul(hl, to, tanhc)
                return hl, tanhc

            def allgather(hl, l):
                agin = dram.tile([P, B], F32R, tag=f"agin{l}", name=f"agin{l}")
                agout = dram.tile([H, B], F32R, tag=f"agout{l}", name=f"agout{l}")
                nc.sync.dma_start(out=agin, in_=hl)
                nc.gpsimd.collective_compute(
                    "AllGather",
                    mybir.AluOpType.bypass,
                    replica_groups=GROUPS,
                    ins=[agin.opt()],
                    outs=[agout.opt()],
                )
                hT = acts.tile([P, KT_H, B], F32R, tag=f"hT{l}", name=f"hT{l}")
                ck = 2  # k-tiles per gather chunk
                for c0 in range(0, KT_H, ck):
                    nc.sync.dma_start(
                        out=hT[:, c0:c0 + ck, :],
                        in_=agout[c0 * P:(c0 + ck) * P, :].rearrange("(kk p) b -> p kk b", p=P))
                return hT

            # rolling state across the unrolled time loop
            enc_t = acts.tile([P, KT_H, B], F32R, tag="encT", name="encT")
            nc.sync.dma_start(out=enc_t, in_=enc0[:].rearrange("(kk p) b -> p kk b", p=P))
            hT = [None] * L        # gathered h_l(t) feature-major
            pg_cur = [None] * L    # psum tiles pre-loaded with W_hh @ h_l(t-1)
            rT_prev = None         # r(t-1) for the deferred batch-major output write
            t_prev = None

            def emit_outbt(rT, tstep):
                """out[:, tstep, :] = (Wd2 @ r)^T via lhsT=rT; bias added on host."""
                ob = obuf.tile([P, B // P, D_IN], F32, tag="ob", name="ob")
                for m in range(B // P):
                    po = pwork.tile([P, D_IN], F32, tag="pw", name="pw")
                    for kk in range(KT_H):
                        nc.tensor.matmul(
                            po,
                            lhsT=(rT[:, kk, m * P:(m + 1) * P]),
                            rhs=(s_wd2[:, kk, :]),
                            start=kk == 0,
                            stop=kk == KT_H - 1,
                        )
                    nc.vector.tensor_copy(out=ob[:, m, :], in_=po)
                nc.sync.dma_start(
                    out=out[:, tstep, :].rearrange("(c p) d -> p c d", p=P),
                    in_=ob,
                )

            for t in range(timesteps):
                first = t == 0

                # ---- layer 0 ----
                if first:
                    pg_cur[0] = pgp.tile([P, MT_G, B], F32, tag="pg0", name="pg0")
                gate_mms(pg_cur[0], s_wih[0], enc_t, first=first, last=True)
                h0l, tanhc0 = elementwise(0, pg_cur[0], first)
                hT0_new = allgather(h0l, 0)
                # AG0 window: W_hh[2] @ h2(t-1) for THIS step's L2, plus the
                # deferred batch-major output write of step t-1.
                if not first:
                    _pe_touch(nc, tanhc2_prev)
                    pg_cur[2] = pgp.tile([P, MT_G, B], F32, tag="pg2", name="pg2")
                    gate_mms(pg_cur[2], s_whh[2], hT[2], first=True, last=False)
                    emit_outbt(rT_prev, t_prev)
                hT[0] = hT0_new

                # ---- layer 1 ----
                if first:
                    pg_cur[1] = pgp.tile([P, MT_G, B], F32, tag="pg1", name="pg1")
                gate_mms(pg_cur[1], s_wih[1], hT[0], first=first, last=True)
                h1l, tanhc1 = elementwise(1, pg_cur[1], first)
                hT1_new = allgather(h1l, 1)
                # AG1 window: W_hh[0] @ h0(t) for step t+1.
                if t + 1 < timesteps:
                    _pe_touch(nc, tanhc0)
                    pg_cur[0] = pgp.tile([P, MT_G, B], F32, tag="pg0", name="pg0")
                    gate_mms(pg_cur[0], s_whh[0], hT[0], first=True, last=False)
                hT[1] = hT1_new

                # ---- layer 2 ----
                if first:
                    pg_cur[2] = pgp.tile([P, MT_G, B], F32, tag="pg2", name="pg2")
                gate_mms(pg_cur[2], s_wih[2], hT[1], first=first, last=True)
                h2l, tanhc2 = elementwise(2, pg_cur[2], first)
                hT2_new = allgather(h2l, 2)
                _pe_touch(nc, hT2_new[:, 0, :])
                # AG2 window: W_hh[1] @ h1(t) for step t+1.
                if t + 1 < timesteps:
                    _pe_touch(nc, tanhc1)
                    pg_cur[1] = pgp.tile([P, MT_G, B], F32, tag="pg1", name="pg1")
                    gate_mms(pg_cur[1], s_whh[1], hT[1], first=True, last=False)
                hT[2] = hT2_new

                # ---- decoder (replicated on every core) ----
                rT = acts.tile([P, KT_H, B], F32R, tag="rT", name="rT")
                for m in range(KT_H):
                    pd = pwork.tile([P, B], F32, tag="pw", name="pw")
                    for kk in range(KT_H):
                        nc.tensor.matmul(
                            pd,
                            lhsT=(s_wd1[:, kk, m * P:(m + 1) * P]),
                            rhs=(hT[2][:, kk, :]),
                            start=kk == 0,
                            stop=kk == KT_H - 1,
                        )
                    nc.scalar.activation(rT[:, m, :], pd, AF.Relu, bias=s_bd1[:, m:m + 1])

                if t + 1 < timesteps:
                    # outT = Wd2 @ rT + bd2 (feature-major, feeds re-encode)
                    outT = acts.tile([P, KT_D, B], F32R, tag="outT", name="outT")
                    for m in range(KT_D):
                        pd = pwork.tile([P, B], F32, tag="pw", name="pw")
                        for kk in range(KT_H):
                            nc.tensor.matmul(
                                pd,
                                lhsT=(s_wd2[:, kk, m * P:(m + 1) * P]),
                                rhs=(rT[:, kk, :]),
                                start=kk == 0,
                                stop=kk == KT_H - 1,
                            )
                        nc.scalar.add(outT[:, m, :], pd, add=s_bd2[:, m:m + 1])
                    # enc(t+1) = relu(We @ outT + be)
                    enc_t = acts.tile([P, KT_H, B], F32R, tag="encT", name="encT")
                    for m in range(KT_H):
                        pd = pwork.tile([P, B], F32, tag="pw", name="pw")
                        for kk in range(KT_D):
                            nc.tensor.matmul(
                                pd,
                                lhsT=(s_we[:, kk, m * P:(m + 1) * P]),
                                rhs=(outT[:, kk, :]),
                                start=kk == 0,
                                stop=kk == KT_D - 1,
                            )
                        nc.scalar.activation(enc_t[:, m, :], pd, AF.Relu, bias=s_be[:, m:m + 1])
                    rT_prev, t_prev = rT, t
                    tanhc2_prev = tanhc2
                else:
                    emit_outbt(rT, t)

    nc.compile()
    return nc


_CACHE = {}


def _get_program(timesteps):
    if timesteps not in _CACHE:
        _CACHE[timesteps] = build_program(timesteps)
    return _CACHE[timesteps]


def _prep_inputs(x, We, be, W_ih, W_hh, b_ih, b_hh, Wd1, bd1, Wd2, bd2):
    """Host-side layout: shard/transpose weights per core, fold biases."""
    f = np.float32
    x, We, be = np.asarray(x, f), np.asarray(We, f), np.asarray(be, f)
    W_ih, W_hh = np.asarray(W_ih, f), np.asarray(W_hh, f)
    b_ih, b_hh = np.asarray(b_ih, f), np.asarray(b_hh, f)
    Wd1, bd1 = np.asarray(Wd1, f), np.asarray(bd1, f)
    Wd2, bd2 = np.asarray(Wd2, f), np.asarray(bd2, f)

    enc0T = np.ascontiguousarray(np.maximum(x @ We.T + be, 0.0).T)  # [H, B]
    wd1T = np.ascontiguousarray(Wd1.T)
    wd2T = np.ascontiguousarray(Wd2.T)
    weT = np.ascontiguousarray(We.T)
    bd1c = np.ascontiguousarray(bd1.reshape(KT_H, P).T)
    bd2c = np.ascontiguousarray(bd2.reshape(KT_D, P).T)
    bec = np.ascontiguousarray(be.reshape(KT_H, P).T)

    in_maps = []
    for k in range(NCORES):
        rows = np.concatenate(
            [np.arange(g * H + k * SH, g * H + (k + 1) * SH) for g in range(4)]
        )
        m = {
            "wd1": wd1T, "wd2": wd2T, "we": weT,
            "bd1": bd1c, "bd2": bd2c, "be": bec, "enc0": enc0T,
        }
        for l in range(L):
            m[f"wih{l}"] = np.ascontiguousarray(W_ih[l][rows, :].T)
            m[f"whh{l}"] = np.ascontiguousarray(W_hh[l][rows, :].T)
            bsum = (b_ih[l] + b_hh[l])[rows]
            m[f"bg{l}"] = np.ascontiguousarray(bsum.reshape(MT_G, SH).T)
        in_maps.append(m)
    return in_maps, bd2


def kernel(x, We, be, W_ih, W_hh, b_ih, b_hh, Wd1, bd1, Wd2, bd2, timesteps, **run_kw):
    tsteps = int(timesteps)
    nc = _get_program(tsteps)
    in_maps, bd2_np = _prep_inputs(x, We, be, W_ih, W_hh, b_ih, b_hh, Wd1, bd1, Wd2, bd2)
    res = run_bass_kernel_spmd(nc, in_maps, core_ids=list(range(NCORES)), **run_kw)
    kernel.last_results = res
    out = np.asarray(res.results[0]["out"], np.float32) + bd2_np[None, None, :]
    return out



# revision 2
# speedup vs baseline: 1.4842x; 1.4842x over previous
"""Trainium2 Bass kernel for the DigitalTwinModel (3-layer LSTM digital twin).

Strategy: hybrid MP-4 x DP-2 in bf16.
  - The 8 cores form two replica groups {0..3} and {4..7}; each group owns a
    batch half (128 rows).  Within a group the hidden dim is sharded 4-way:
    core (g, r) owns hidden features r*256:(r+1)*256 of every LSTM layer's
    h/c state plus the matching 4x256 gate rows of W_ih/W_hh.
  - Everything is bf16 on the wire and in the PE (fp32 PSUM accumulate,
    fp32 elementwise/cell state), which halves SBUF so a 1/4 weight shard
    fits resident, halves AllGather payloads ([1024,128] bf16 = 256KB
    output), and runs the PE at full rate at batch=128 free size.
  - 3 AllGathers per timestep (one per layer, 4-rank groups).  The decoder
    (Wd1+relu, then the folded matrix M = We@Wd2 which fuses the output
    projection with the re-encode) is replicated per core; out[:,t,:] is
    produced batch-major via matmul(lhsT=rT, rhs=Wd2^T) one step deferred,
    inside the next step's AG0 window, together with the W_hh prefetches
    for the next step's gate PSUMs.
  - Gate biases (and decoder biases) are folded into PSUM with rank-1
    "ones" matmuls so the gate activations can run as two wide ops
    (sigmoid over i,f,o; tanh over g) instead of eight biased ones.
  - PSUM budget: three [128,8,128] fp32 gate accumulators (2 banks each);
    the decoder reuses pg1 (d1) and pg2 (enc) and the out-write reuses a
    2KB view of pg0, all in dead windows of the gate lifetimes.
"""

import numpy as np
import ml_dtypes

import concourse.bass as bass
import concourse.mybir as mybir
from concourse import bacc
import concourse.tile as tile
from concourse.bass_utils import run_bass_kernel_spmd

F32 = mybir.dt.float32
BF16 = mybir.dt.bfloat16
AF = mybir.ActivationFunctionType

B, D_IN, H, L, T = 256, 512, 1024, 3, 32
NCORES = 8
GP = 4                     # ranks per replica group
NG = NCORES // GP          # replica groups (data-parallel)
BH = B // NG               # batch rows per group
P = 128
SH = H // GP               # hidden features owned per core (256)
KT_H = H // P              # 8 k-tiles over the hidden dim
MT_G = 4 * SH // P         # 8 m-tiles of gates per core
GROUPS = [[0, 1, 2, 3], [4, 5, 6, 7]]
# gate m-tile order: (g,g,i,i,f,f,o,o) so tanh(g) can start while the
# remaining gate m-tiles are still in the matmul pipeline.
GATE_ORDER = [2, 0, 1, 3]  # torch gate chunks: i=0, f=1, g=2, o=3


def _touch(nc, ap2d):
    """Tiny ldweights that makes the PE observe a tile's producer semaphore
    (fused matmuls have a single sync-wait slot; see baseline note)."""
    nc.tensor.ldweights(weights=ap2d[0:1, 0:2].bitcast(BF16))


def build_program(timesteps=T):
    nc = bacc.Bacc(None, num_devices=NCORES, dynamic_dma_scratch_size=2048)

    # ---- kernel I/O (per-core payloads supplied from the host) ----
    wih = [nc.dram_tensor(f"wih{l}", [H, 4 * SH], BF16, kind="ExternalInput") for l in range(L)]
    whh = [nc.dram_tensor(f"whh{l}", [H, 4 * SH], BF16, kind="ExternalInput") for l in range(L)]
    bg = [nc.dram_tensor(f"bg{l}", [1, 4 * SH], BF16, kind="ExternalInput") for l in range(L)]
    wd1 = nc.dram_tensor("wd1", [H, H], BF16, kind="ExternalInput")
    wm = nc.dram_tensor("wm", [H, H], BF16, kind="ExternalInput")
    wd2 = nc.dram_tensor("wd2", [H, D_IN], BF16, kind="ExternalInput")
    bd1 = nc.dram_tensor("bd1", [1, H], BF16, kind="ExternalInput")
    bm = nc.dram_tensor("bm", [1, H], BF16, kind="ExternalInput")
    enc0 = nc.dram_tensor("enc0", [H, BH], BF16, kind="ExternalInput")
    out = nc.dram_tensor("out", [BH, timesteps, D_IN], F32, kind="ExternalOutput")

    with tile.TileContext(nc) as tc:
        with (
            tc.tile_pool(name="singles", bufs=1) as singles,
            tc.tile_pool(name="acts", bufs=1) as acts,
            tc.tile_pool(name="gtmp", bufs=1) as gtmp,
            tc.tile_pool(name="hloc", bufs=2) as hlocp,
            tc.tile_pool(name="obuf", bufs=1) as obuf,
            tc.tile_pool(name="pg", bufs=1, space="PSUM") as pgp,
            tc.tile_pool(name="dram", bufs=2, space="DRAM") as dram,
        ):
            # ---- load resident weights/biases into SBUF ----
            s_wih, s_whh, s_bg = [], [], []
            for l in range(L):
                w = singles.tile([P, KT_H, 4 * SH], BF16, tag=f"swih{l}", name=f"swih{l}")
                nc.sync.dma_start(out=w, in_=wih[l][:].rearrange("(kk p) m -> p kk m", p=P))
                _touch(nc, w[:, 0, :])
                s_wih.append(w)
            for l in range(L):
                w = singles.tile([P, KT_H, 4 * SH], BF16, tag=f"swhh{l}", name=f"swhh{l}")
                nc.sync.dma_start(out=w, in_=whh[l][:].rearrange("(kk p) m -> p kk m", p=P))
                _touch(nc, w[:, 0, :])
                s_whh.append(w)
            for l in range(L):
                t_ = singles.tile([1, 4 * SH], BF16, tag=f"sbg{l}", name=f"sbg{l}")
                nc.sync.dma_start(out=t_, in_=bg[l][:])
                s_bg.append(t_)
            s_wd1 = singles.tile([P, KT_H, H], BF16, tag="swd1", name="swd1")
            nc.sync.dma_start(out=s_wd1, in_=wd1[:].rearrange("(kk p) m -> p kk m", p=P))
            _touch(nc, s_wd1[:, 0, :])
            s_wm = singles.tile([P, KT_H, H], BF16, tag="swm", name="swm")
            nc.sync.dma_start(out=s_wm, in_=wm[:].rearrange("(kk p) m -> p kk m", p=P))
            _touch(nc, s_wm[:, 0, :])
            s_wd2 = singles.tile([P, KT_H, D_IN], BF16, tag="swd2", name="swd2")
            nc.sync.dma_start(out=s_wd2, in_=wd2[:].rearrange("(kk p) m -> p kk m", p=P))
            _touch(nc, s_wd2[:, 0, :])
            s_bd1 = singles.tile([1, H], BF16, tag="sbd1", name="sbd1")
            nc.sync.dma_start(out=s_bd1, in_=bd1[:])
            s_bm = singles.tile([1, H], BF16, tag="sbm", name="sbm")
            nc.sync.dma_start(out=s_bm, in_=bm[:])
            ones = singles.tile([1, BH], BF16, tag="ones", name="ones")
            nc.vector.memset(ones, 1.0)

            # persistent cell state (zero-initialised), [256 features] as [P,2,BH]
            s_c = []
            for l in range(L):
                c = singles.tile([P, 2, BH], F32, tag=f"c{l}", name=f"c{l}")
                nc.vector.memset(c, 0.0)
                s_c.append(c)

            def bias_fold(pg_t, btile, n_mt):
                """Open the accumulation groups of pg_t's banks and add the
                per-(partition, m-tile) bias via a rank-1 ones matmul.
                start=True on the bank-first m-tile clears the whole 2KiB
                bank (4 m-tiles); later writes land in accumulate mode."""
                for m in range(n_mt):
                    nc.tensor.matmul(
                        pg_t[:, m, :],
                        lhsT=btile[0:1, m * P:(m + 1) * P],
                        rhs=ones[0:1, :],
                        start=(m % 4 == 0),
                        stop=False,
                    )

            def mm_acc(pg_t, w, rhs, close, n_mt=MT_G):
                """Accumulate w^T @ rhs into pg_t's m-tiles, m-outer so the
                elementwise can chase m-tiles. If close, the bank-last
                m-tile's final k closes the accumulation group."""
                for m in range(n_mt):
                    for kk in range(KT_H):
                        nc.tensor.matmul(
                            pg_t[:, m, :],
                            lhsT=w[:, kk, m * P:(m + 1) * P],
                            rhs=rhs[:, kk, :],
                            start=False,
                            stop=(close and kk == KT_H - 1 and m % 4 == 3),
                        )

            def elementwise(l, pg_t):
                """gates (g,g,i,i,f,f,o,o) -> h'_l slice (bf16), update c."""
                tg = gtmp.tile([P, 2, BH], F32, tag="tg", name="tg")
                s_if = gtmp.tile([P, 4, BH], F32, tag="sif", name="sif")
                s_o = gtmp.tile([P, 2, BH], F32, tag="so", name="so")
                t1 = gtmp.tile([P, 2, BH], F32, tag="t1", name="t1")
                t2 = gtmp.tile([P, 2, BH], F32, tag="t2", name="t2")
                tc_ = gtmp.tile([P, 2, BH], F32, tag="tc", name="tc")
                nc.scalar.activation(tg, pg_t[:, 0:2, :], AF.Tanh)
                nc.scalar.activation(s_if, pg_t[:, 2:6, :], AF.Sigmoid)
                nc.vector.tensor_mul(t1, s_if[:, 0:2, :], tg)      # i * g
                nc.vector.tensor_mul(t2, s_if[:, 2:4, :], s_c[l])  # f * c
                nc.vector.tensor_add(s_c[l], t1, t2)
                nc.scalar.activation(tc_, s_c[l], AF.Tanh)
                nc.scalar.activation(s_o, pg_t[:, 6:8, :], AF.Sigmoid)
                hl = hlocp.tile([P, 2, BH], BF16, tag=f"hl{l}", name=f"hl{l}")
                nc.vector.tensor_mul(hl, s_o, tc_)
                return hl

            def allgather(hl, l):
                agin = dram.tile([SH, BH], BF16, tag=f"agin{l}", name=f"agin{l}")
                agout = dram.tile([H, BH], BF16, tag=f"agout{l}", name=f"agout{l}")
                nc.sync.dma_start(out=agin[:].rearrange("(c p) b -> p c b", p=P), in_=hl)
                nc.gpsimd.collective_compute(
                    "AllGather",
                    mybir.AluOpType.bypass,
                    replica_groups=GROUPS,
                    ins=[agin.opt()],
                    outs=[agout.opt()],
                )
                return agout

            def fetch_hT(agout, l):
                hT = acts.tile([P, KT_H, BH], BF16, tag=f"hT{l}", name=f"hT{l}")
                nc.sync.dma_start(out=hT, in_=agout[:].rearrange("(kk p) b -> p kk b", p=P))
                return hT

            def emit_outbt(rT, tstep):
                """out[:, tstep, :] = (rT^T @ Wd2^T); bd2 added on host.
                Uses a 2KB view of pg0's bank as the PSUM target."""
                _touch(nc, rT[:, 0, :])
                po = pgp.tile([P, MT_G, BH], F32, tag="pg0", name="po")
                pov = po[:, 0:4, :].rearrange("p a b -> p (a b)")
                for kk in range(KT_H):
                    nc.tensor.matmul(
                        pov,
                        lhsT=rT[:, kk, :],
                        rhs=s_wd2[:, kk, :],
                        start=kk == 0,
                        stop=kk == KT_H - 1,
                    )
                ob = obuf.tile([P, D_IN], F32, tag="ob", name="ob")
                nc.vector.tensor_copy(out=ob, in_=pov)
                nc.sync.dma_start(out=out[:, tstep, :], in_=ob)

            # ---- prologue: enc(0) and opened gate groups (h(-1) = 0) ----
            enc_t = acts.tile([P, KT_H, BH], BF16, tag="encT", name="encT")
            nc.sync.dma_start(out=enc_t, in_=enc0[:].rearrange("(kk p) b -> p kk b", p=P))
            pg = [None] * L
            for l in range(L):
                pg[l] = pgp.tile([P, MT_G, BH], F32, tag=f"pg{l}", name=f"pg{l}")
                bias_fold(pg[l], s_bg[l], MT_G)

            hT = [None] * L
            rT_prev, t_prev = None, None

            for t in range(timesteps):
                # ---- layer 0 gates (enc_t ready from prologue / dec chain) ----
                mm_acc(pg[0], s_wih[0], enc_t, close=True)
                h0l = elementwise(0, pg[0])
                ag0 = allgather(h0l, 0)
                # AG0 window: deferred out-write of t-1, W_hh1/W_hh2 prefetch
                if t > 0:
                    emit_outbt(rT_prev, t_prev)
                    _touch(nc, hT[1][:, 0, :])
                    pg[1] = pgp.tile([P, MT_G, BH], F32, tag="pg1", name="pg1")
                    bias_fold(pg[1], s_bg[1], MT_G)
                    mm_acc(pg[1], s_whh[1], hT[1], close=False)
                    _touch(nc, hT[2][:, 0, :])
                    pg[2] = pgp.tile([P, MT_G, BH], F32, tag="pg2", name="pg2")
                    bias_fold(pg[2], s_bg[2], MT_G)
                    mm_acc(pg[2], s_whh[2], hT[2], close=False)
                hT[0] = fetch_hT(ag0, 0)

                # ---- layer 1 ----
                _touch(nc, hT[0][:, 0, :])
                mm_acc(pg[1], s_wih[1], hT[0], close=True)
                h1l = elementwise(1, pg[1])
                ag1 = allgather(h1l, 1)
                # AG1 window: W_hh0 prefetch for t+1
                if t + 1 < timesteps:
                    pg[0] = pgp.tile([P, MT_G, BH], F32, tag="pg0", name="pg0")
                    bias_fold(pg[0], s_bg[0], MT_G)
                    mm_acc(pg[0], s_whh[0], hT[0], close=False)
                hT[1] = fetch_hT(ag1, 1)

                # ---- layer 2 ----
                _touch(nc, hT[1][:, 0, :])
                mm_acc(pg[2], s_wih[2], hT[1], close=True)
                h2l = elementwise(2, pg[2])
                ag2 = allgather(h2l, 2)
                hT[2] = fetch_hT(ag2, 2)

                # ---- decoder chain: d1 -> rT -> (M -> enc -> G0) ----
                _touch(nc, hT[2][:, 0, :])
                pd1 = pgp.tile([P, MT_G, BH], F32, tag="pg1", name="pd1")
                bias_fold(pd1, s_bd1, MT_G)
                mm_acc(pd1, s_wd1, hT[2], close=True)
                rT = acts.tile([P, KT_H, BH], BF16, tag="rT", name="rT")
                for m2 in range(0, MT_G, 2):
                    nc.scalar.activation(
                        rT[:, m2:m2 + 2, :], pd1[:, m2:m2 + 2, :], AF.Relu)

                if t + 1 < timesteps:
                    _touch(nc, rT[:, 0, :])
                    pe_ = pgp.tile([P, MT_G, BH], F32, tag="pg2", name="pe")
                    bias_fold(pe_, s_bm, MT_G)
                    mm_acc(pe_, s_wm, rT, close=True)
                    enc_t = acts.tile([P, KT_H, BH], BF16, tag="encT", name="encT")
                    nc.scalar.activation(enc_t, pe_, AF.Relu)
                    rT_prev, t_prev = rT, t
                else:
                    emit_outbt(rT, t)

    nc.compile()
    return nc


_CACHE = {}


def _get_program(timesteps):
    if timesteps not in _CACHE:
        _CACHE[timesteps] = build_program(timesteps)
    return _CACHE[timesteps]


def _prep_inputs(x, We, be, W_ih, W_hh, b_ih, b_hh, Wd1, bd1, Wd2, bd2):
    """Host-side layout: bf16 weights, per-core gate-row shards, folded
    decoder matrix M = We@Wd2, batch halves per replica group."""
    f = np.float32
    bf = ml_dtypes.bfloat16
    x, We, be = np.asarray(x, f), np.asarray(We, f), np.asarray(be, f)
    W_ih, W_hh = np.asarray(W_ih, f), np.asarray(W_hh, f)
    b_ih, b_hh = np.asarray(b_ih, f), np.asarray(b_hh, f)
    Wd1, bd1 = np.asarray(Wd1, f), np.asarray(bd1, f)
    Wd2, bd2 = np.asarray(Wd2, f), np.asarray(bd2, f)

    enc0T = np.maximum(x @ We.T + be, 0.0).T          # [H, B]
    M = We @ Wd2                                       # [H, H]
    bM = We @ bd2 + be                                 # [H]

    wd1T = np.ascontiguousarray(Wd1.T).astype(bf)
    wmT = np.ascontiguousarray(M.T).astype(bf)
    wd2T = np.ascontiguousarray(Wd2.T).astype(bf)
    bd1c = np.ascontiguousarray(bd1[None, :]).astype(bf)
    bmc = np.ascontiguousarray(bM[None, :]).astype(bf)

    in_maps = []
    for k in range(NCORES):
        g, r = k // GP, k % GP
        rows = np.concatenate(
            [np.arange(q * H + r * SH, q * H + (r + 1) * SH) for q in GATE_ORDER]
        )
        m = {
            "wd1": wd1T, "wm": wmT, "wd2": wd2T, "bd1": bd1c, "bm": bmc,
            "enc0": np.ascontiguousarray(enc0T[:, g * BH:(g + 1) * BH]).astype(bf),
        }
        for l in range(L):
            m[f"wih{l}"] = np.ascontiguousarray(W_ih[l][rows, :].T).astype(bf)
            m[f"whh{l}"] = np.ascontiguousarray(W_hh[l][rows, :].T).astype(bf)
            bsum = (b_ih[l] + b_hh[l])[rows]
            m[f"bg{l}"] = np.ascontiguousarray(bsum[None, :]).astype(bf)
        in_maps.append(m)
    return in_maps, bd2


def kernel(x, We, be, W_ih, W_hh, b_ih, b_hh, Wd1, bd1, Wd2, bd2, timesteps, **run_kw):
    tsteps = int(timesteps)
    nc = _get_program(tsteps)
    in_maps, bd2_np = _prep_inputs(x, We, be, W_ih, W_hh, b_ih, b_hh, Wd1, bd1, Wd2, bd2)
    res = run_bass_kernel_spmd(nc, in_maps, core_ids=list(range(NCORES)), **run_kw)
    kernel.last_results = res
    halves = [np.asarray(res.results[g * GP]["out"], np.float32) for g in range(NG)]
    out = np.concatenate(halves, axis=0) + bd2_np[None, None, :]
    return out


# revision 9
# speedup vs baseline: 1.5772x; 1.0627x over previous
"""Trainium2 Bass kernel for the DigitalTwinModel (3-layer LSTM digital twin).

Strategy: hybrid MP-4 x DP-2 in bf16.
  - The 8 cores form two replica groups {0..3} and {4..7}; each group owns a
    batch half (128 rows).  Within a group the hidden dim is sharded 4-way:
    core (g, r) owns hidden features r*256:(r+1)*256 of every LSTM layer's
    h/c state plus the matching 4x256 gate rows of W_ih/W_hh.
  - Everything is bf16 on the wire and in the PE (fp32 PSUM accumulate,
    fp32 elementwise/cell state): a 1/4 weight shard fits SBUF-resident,
    AllGather payloads halve ([1024,128] bf16 out = 256KB -> ~21.5us), and
    the PE runs at full rate at batch=128 free size.
  - 3 AllGathers per timestep (one per layer, 4-rank groups).  The decoder
    (Wd1+relu, then M = We@Wd2 which fuses the output projection with the
    re-encode) is replicated per core; out[:,t,:] is produced batch-major
    via matmul(lhsT=rT, rhs=Wd2^T) one step deferred inside the next AG0
    window, together with the W_hh prefetches into the gate PSUMs.
  - Gate PSUM is split per 2KB bank (pgXa = g,g,i,i / pgXb = f,f,o,o) so
    the elementwise chain can start as soon as the first bank's
    accumulation group closes instead of waiting for all 64 matmuls.
    Decoder PSUMs reuse the gate banks in dead lifetime windows.
  - Gather buffers use a [P, rank*(2B)] layout so DMA runs are 512B and
    dodge the sub-512B descriptor penalty.
  - Tunable filler matmuls on resident weights keep the PE p-state ramped
    through the collective windows so every critical-path matmul burst
    runs at full clock.
"""

import numpy as np
import ml_dtypes

import concourse.bass as bass
import concourse.mybir as mybir
from concourse import bacc
import concourse.tile as tile
from concourse.bass_utils import run_bass_kernel_spmd

F32 = mybir.dt.float32
BF16 = mybir.dt.bfloat16
AF = mybir.ActivationFunctionType

B, D_IN, H, L, T = 256, 512, 1024, 3, 32
NCORES = 8
GP = 4                     # ranks per replica group
NG = NCORES // GP          # replica groups (data-parallel)
BH = B // NG               # batch rows per group
P = 128
SH = H // GP               # hidden features owned per core (256)
KT_H = H // P              # 8 k-tiles over the hidden dim
MT_G = 4 * SH // P         # 8 m-tiles of gates per core
HB = MT_G // 2             # 4 m-tiles per PSUM bank
GROUPS = [[0, 1, 2, 3], [4, 5, 6, 7]]
# gate m-tile order: (g,g,i,i | f,f,o,o) -> bank A holds g,i; bank B f,o.
GATE_ORDER = [2, 0, 1, 3]  # torch gate chunks: i=0, f=1, g=2, o=3

# PE p-state filler matmuls per collective window (each ~213ns at 2.4GHz)
FILL = (78, 90, 96)


def _touch(nc, ap2d):
    """Tiny ldweights that makes the PE observe a tile's producer semaphore
    (fused matmuls have a single sync-wait slot)."""
    nc.tensor.ldweights(weights=ap2d[0:1, 0:2].bitcast(BF16))


def build_program(timesteps=T):
    nc = bacc.Bacc(None, num_devices=NCORES, dynamic_dma_scratch_size=2048)

    # ---- kernel I/O (per-core payloads supplied from the host) ----
    wih = [nc.dram_tensor(f"wih{l}", [H, 4 * SH], BF16, kind="ExternalInput") for l in range(L)]
    whh = [nc.dram_tensor(f"whh{l}", [H, 4 * SH], BF16, kind="ExternalInput") for l in range(L)]
    bg = [nc.dram_tensor(f"bg{l}", [1, 4 * SH], BF16, kind="ExternalInput") for l in range(L)]
    wd1 = nc.dram_tensor("wd1", [H, H], BF16, kind="ExternalInput")
    wm = nc.dram_tensor("wm", [H, H], BF16, kind="ExternalInput")
    wd2 = nc.dram_tensor("wd2", [H, D_IN], BF16, kind="ExternalInput")
    bd1 = nc.dram_tensor("bd1", [1, H], BF16, kind="ExternalInput")
    bm = nc.dram_tensor("bm", [1, H], BF16, kind="ExternalInput")
    enc0 = nc.dram_tensor("enc0", [H, BH], BF16, kind="ExternalInput")
    out = nc.dram_tensor("out", [BH, timesteps, D_IN], F32, kind="ExternalOutput")

    with tile.TileContext(nc) as tc:
        with (
            tc.tile_pool(name="singles", bufs=1) as singles,
            tc.tile_pool(name="acts", bufs=1) as acts,
            tc.tile_pool(name="gtmp", bufs=1) as gtmp,
            tc.tile_pool(name="hloc", bufs=2) as hlocp,
            tc.tile_pool(name="obuf", bufs=1) as obuf,
            tc.tile_pool(name="pg", bufs=1, space="PSUM") as pgp,
            tc.tile_pool(name="dram", bufs=2, space="DRAM") as dram,
        ):
            # ---- load resident weights/biases into SBUF ----
            s_wih, s_whh, s_bg = [], [], []
            for l in range(L):
                w = singles.tile([P, KT_H, 4 * SH], BF16, tag=f"swih{l}", name=f"swih{l}")
                nc.sync.dma_start(out=w, in_=wih[l][:].rearrange("(kk p) m -> p kk m", p=P))
                _touch(nc, w[:, 0, :])
                s_wih.append(w)
            for l in range(L):
                w = singles.tile([P, KT_H, 4 * SH], BF16, tag=f"swhh{l}", name=f"swhh{l}")
                nc.sync.dma_start(out=w, in_=whh[l][:].rearrange("(kk p) m -> p kk m", p=P))
                _touch(nc, w[:, 0, :])
                s_whh.append(w)
            for l in range(L):
                t_ = singles.tile([1, 4 * SH], BF16, tag=f"sbg{l}", name=f"sbg{l}")
                nc.sync.dma_start(out=t_, in_=bg[l][:])
                s_bg.append(t_)
            s_wd1 = singles.tile([P, KT_H, H], BF16, tag="swd1", name="swd1")
            nc.sync.dma_start(out=s_wd1, in_=wd1[:].rearrange("(kk p) m -> p kk m", p=P))
            _touch(nc, s_wd1[:, 0, :])
            s_wm = singles.tile([P, KT_H, H], BF16, tag="swm", name="swm")
            nc.sync.dma_start(out=s_wm, in_=wm[:].rearrange("(kk p) m -> p kk m", p=P))
            _touch(nc, s_wm[:, 0, :])
            s_wd2 = singles.tile([P, KT_H, D_IN], BF16, tag="swd2", name="swd2")
            nc.sync.dma_start(out=s_wd2, in_=wd2[:].rearrange("(kk p) m -> p kk m", p=P))
            _touch(nc, s_wd2[:, 0, :])
            s_bd1 = singles.tile([1, H], BF16, tag="sbd1", name="sbd1")
            nc.sync.dma_start(out=s_bd1, in_=bd1[:])
            s_bm = singles.tile([1, H], BF16, tag="sbm", name="sbm")
            nc.sync.dma_start(out=s_bm, in_=bm[:])
            ones = singles.tile([1, BH], BF16, tag="ones", name="ones")
            nc.vector.memset(ones, 1.0)

            s_c = []
            for l in range(L):
                c = singles.tile([P, 2, BH], F32, tag=f"c{l}", name=f"c{l}")
                nc.vector.memset(c, 0.0)
                s_c.append(c)

            def pghalf(tag, name):
                return pgp.tile([P, HB, BH], F32, tag=tag, name=name)

            def bias_fold(pa, pb, btile):
                """Open both banks' accumulation groups; add per-(partition,
                m-tile) biases via rank-1 ones matmuls. start=True on the
                bank-first m-tile clears the whole 2KiB bank."""
                for half, pt in ((0, pa), (1, pb)):
                    for m in range(HB):
                        nc.tensor.matmul(
                            pt[:, m, :],
                            lhsT=btile[0:1, (half * HB + m) * P:(half * HB + m + 1) * P],
                            rhs=ones[0:1, :],
                            start=(m == 0),
                            stop=False,
                        )

            def rhs_kt(hT, kk):
                """k-tile kk of a gathered tensor in [P, GP, 2*BH]... note:
                hT here is [P, GP, 2, BH]; kk maps to (rank, half)."""
                return hT[:, kk // 2, kk % 2, :]

            def mm_gates(pa, pb, w, hT, close):
                """Accumulate w^T @ hT into the two bank tiles, bank-A m-tiles
                first so elementwise can start while bank B accumulates."""
                for half, pt in ((0, pa), (1, pb)):
                    for m in range(HB):
                        for kk in range(KT_H):
                            nc.tensor.matmul(
                                pt[:, m, :],
                                lhsT=w[:, kk, (half * HB + m) * P:(half * HB + m + 1) * P],
                                rhs=rhs_kt(hT, kk),
                                start=False,
                                stop=(close and kk == KT_H - 1 and m == HB - 1),
                            )

            def mm_dense(pa, pb, w, xa, xb, close, kk_outer=False):
                """Like mm_gates but rhs is a local tensor split into two
                [P, HB, BH] half tiles. kk_outer=True consumes the halves
                incrementally (for chains where they become ready in order)."""
                def xkt(kk):
                    return (xa if kk < HB else xb)[:, kk % HB, :]
                if kk_outer:
                    for phase in range(2):
                        for kk in range(phase * HB, (phase + 1) * HB):
                            for half, pt in ((0, pa), (1, pb)):
                                for m in range(HB):
                                    nc.tensor.matmul(
                                        pt[:, m, :],
                                        lhsT=w[:, kk, (half * HB + m) * P:(half * HB + m + 1) * P],
                                        rhs=xkt(kk),
                                        start=False,
                                        stop=(close and kk == KT_H - 1 and m == HB - 1),
                                    )
                else:
                    for half, pt in ((0, pa), (1, pb)):
                        for m in range(HB):
                            for kk in range(KT_H):
                                nc.tensor.matmul(
                                    pt[:, m, :],
                                    lhsT=w[:, kk, (half * HB + m) * P:(half * HB + m + 1) * P],
                                    rhs=xkt(kk),
                                    start=False,
                                    stop=(close and kk == KT_H - 1 and m == HB - 1),
                                )

            def elementwise(l, pa, pb):
                """bank A = (g,g,i,i), bank B = (f,f,o,o) -> h'_l (bf16)."""
                tg = gtmp.tile([P, 2, BH], F32, tag="tg", name="tg")
                si = gtmp.tile([P, 2, BH], F32, tag="si", name="si")
                sfo = gtmp.tile([P, 4, BH], F32, tag="sfo", name="sfo")
                t1 = gtmp.tile([P, 2, BH], F32, tag="t1", name="t1")
                t2 = gtmp.tile([P, 2, BH], F32, tag="t2", name="t2")
                tc_ = gtmp.tile([P, 2, BH], F32, tag="tc", name="tc")
                nc.scalar.activation(tg, pa[:, 0:2, :], AF.Tanh)
                nc.scalar.activation(si, pa[:, 2:4, :], AF.Sigmoid)
                nc.vector.tensor_mul(t1, si, tg)                   # i * g
                nc.scalar.activation(sfo, pb, AF.Sigmoid)
                nc.vector.tensor_mul(t2, sfo[:, 0:2, :], s_c[l])   # f * c
                nc.vector.tensor_add(s_c[l], t1, t2)
                nc.scalar.activation(tc_, s_c[l], AF.Tanh)
                hl = hlocp.tile([P, 2, BH], BF16, tag=f"hl{l}", name=f"hl{l}")
                nc.vector.tensor_mul(hl, sfo[:, 2:4, :], tc_)
                return hl

            def allgather(hl, l):
                # agin row p = [c0 batch..., c1 batch...]: 512B contiguous
                agin = dram.tile([P, SH // P * BH], BF16, tag=f"agin{l}", name=f"agin{l}")
                agout = dram.tile([GP * P, SH // P * BH], BF16, tag=f"agout{l}", name=f"agout{l}")
                nc.sync.dma_start(out=agin, in_=hl[:].rearrange("p c b -> p (c b)"))
                nc.gpsimd.collective_compute(
                    "AllGather",
                    mybir.AluOpType.bypass,
                    replica_groups=GROUPS,
                    ins=[agin.opt()],
                    outs=[agout.opt()],
                )
                return agout

            def fetch_hT(agout, l):
                # [P, rank, half, BH]; in-side rows are 512B contiguous
                hT = acts.tile([P, GP, 2, BH], BF16, tag=f"hT{l}", name=f"hT{l}")
                nc.sync.dma_start(
                    out=hT[:].rearrange("p r c b -> p r (c b)"),
                    in_=agout[:].rearrange("(r p) x -> p r x", p=P))
                return hT

            def emit_outbt(rTa, rTb, tstep):
                """out[:, tstep, :] = (rT^T @ Wd2^T); bd2 added on host."""
                _touch(nc, rTa[:, 0, :])
                _touch(nc, rTb[:, 0, :])
                po = pghalf("pg0a", "po")
                pov = po[:].rearrange("p a b -> p (a b)")
                for kk in range(KT_H):
                    nc.tensor.matmul(
                        pov,
                        lhsT=(rTa if kk < HB else rTb)[:, kk % HB, :],
                        rhs=s_wd2[:, kk, :],
                        start=kk == 0,
                        stop=kk == KT_H - 1,
                    )
                ob = obuf.tile([P, D_IN], F32, tag="ob", name="ob")
                nc.vector.tensor_copy(out=ob, in_=pov)
                nc.sync.dma_start(out=out[:, tstep, :], in_=ob)

            # ---- PE p-state filler ----
            pfill = pgp.tile([P, D_IN], F32, tag="fill", name="pfill")

            def fill(n):
                for _ in range(n):
                    nc.tensor.matmul(
                        pfill,
                        lhsT=s_wd1[:, 0, 0:P],
                        rhs=s_wd1[:, 1, 0:D_IN],
                        start=True,
                        stop=True,
                        skip_group_check=True,
                    )

            # ---- prologue: enc(0); open gate groups (h(-1) = 0) ----
            enca = acts.tile([P, HB, BH], BF16, tag="enca", name="enca")
            encb = acts.tile([P, HB, BH], BF16, tag="encb", name="encb")
            nc.sync.dma_start(
                out=enca, in_=enc0[0:HB * P, :].rearrange("(kk p) b -> p kk b", p=P))
            nc.sync.dma_start(
                out=encb, in_=enc0[HB * P:H, :].rearrange("(kk p) b -> p kk b", p=P))
            pga = [None] * L
            pgb = [None] * L
            for l in range(L):
                pga[l] = pghalf(f"pg{l}a", f"pg{l}a")
                pgb[l] = pghalf(f"pg{l}b", f"pg{l}b")
                bias_fold(pga[l], pgb[l], s_bg[l])

            hT = [None] * L
            rT_prev, t_prev = None, None

            for t in range(timesteps):
                # ---- layer 0 gates (enc from prologue / dec chain) ----
                mm_dense(pga[0], pgb[0], s_wih[0], enca, encb, close=True,
                         kk_outer=(t > 0))
                h0l = elementwise(0, pga[0], pgb[0])
                ag0 = allgather(h0l, 0)
                # AG0 window: deferred out-write, W_hh1/W_hh2 prefetch
                if t > 0:
                    emit_outbt(rT_prev[0], rT_prev[1], t_prev)
                    _touch(nc, hT[1][:, 0, 0, :])
                    pga[1], pgb[1] = pghalf("pg1a", "pg1a"), pghalf("pg1b", "pg1b")
                    bias_fold(pga[1], pgb[1], s_bg[1])
                    mm_gates(pga[1], pgb[1], s_whh[1], hT[1], close=False)
                    _touch(nc, hT[2][:, 0, 0, :])
                    pga[2], pgb[2] = pghalf("pg2a", "pg2a"), pghalf("pg2b", "pg2b")
                    bias_fold(pga[2], pgb[2], s_bg[2])
                    mm_gates(pga[2], pgb[2], s_whh[2], hT[2], close=False)
                fill(FILL[0])
                hT[0] = fetch_hT(ag0, 0)

                # ---- layer 1 ----
                _touch(nc, hT[0][:, 0, 0, :])
                mm_gates(pga[1], pgb[1], s_wih[1], hT[0], close=True)
                h1l = elementwise(1, pga[1], pgb[1])
                ag1 = allgather(h1l, 1)
                # AG1 window: W_hh0 prefetch for t+1
                if t + 1 < timesteps:
                    pga[0], pgb[0] = pghalf("pg0a", "pg0a"), pghalf("pg0b", "pg0b")
                    bias_fold(pga[0], pgb[0], s_bg[0])
                    mm_gates(pga[0], pgb[0], s_whh[0], hT[0], close=False)
                fill(FILL[1])
                hT[1] = fetch_hT(ag1, 1)

                # ---- layer 2 ----
                _touch(nc, hT[1][:, 0, 0, :])
                mm_gates(pga[2], pgb[2], s_wih[2], hT[1], close=True)
                h2l = elementwise(2, pga[2], pgb[2])
                ag2 = allgather(h2l, 2)
                fill(FILL[2])
                hT[2] = fetch_hT(ag2, 2)

                # ---- decoder chain: d1 -> rT -> (M -> enc -> G0) ----
                _touch(nc, hT[2][:, 0, 0, :])
                pd1a, pd1b = pghalf("pg1a", "pd1a"), pghalf("pg1b", "pd1b")
                bias_fold(pd1a, pd1b, s_bd1)
                mm_gates(pd1a, pd1b, s_wd1, hT[2], close=True)
                rTa = acts.tile([P, HB, BH], BF16, tag="rTa", name="rTa")
                rTb = acts.tile([P, HB, BH], BF16, tag="rTb", name="rTb")
                nc.scalar.activation(rTa, pd1a, AF.Relu)
                nc.scalar.activation(rTb, pd1b, AF.Relu)

                if t + 1 < timesteps:
                    _touch(nc, rTa[:, 0, :])
                    pea, peb = pghalf("pg2a", "pea"), pghalf("pg2b", "peb")
                    bias_fold(pea, peb, s_bm)
                    _touch(nc, rTb[:, 0, :])
                    mm_dense(pea, peb, s_wm, rTa, rTb, close=True, kk_outer=True)
                    enca = acts.tile([P, HB, BH], BF16, tag="enca", name="enca")
                    encb = acts.tile([P, HB, BH], BF16, tag="encb", name="encb")
                    nc.scalar.activation(enca, pea, AF.Relu)
                    nc.scalar.activation(encb, peb, AF.Relu)
                    rT_prev, t_prev = (rTa, rTb), t
                else:
                    emit_outbt(rTa, rTb, t)

    nc.compile()
    return nc


_CACHE = {}


def _get_program(timesteps):
    if timesteps not in _CACHE:
        _CACHE[timesteps] = build_program(timesteps)
    return _CACHE[timesteps]


def _prep_inputs(x, We, be, W_ih, W_hh, b_ih, b_hh, Wd1, bd1, Wd2, bd2):
    """Host-side layout: bf16 weights, per-core gate-row shards, folded
    decoder matrix M = We@Wd2, batch halves per replica group."""
    f = np.float32
    bf = ml_dtypes.bfloat16
    x, We, be = np.asarray(x, f), np.asarray(We, f), np.asarray(be, f)
    W_ih, W_hh = np.asarray(W_ih, f), np.asarray(W_hh, f)
    b_ih, b_hh = np.asarray(b_ih, f), np.asarray(b_hh, f)
    Wd1, bd1 = np.asarray(Wd1, f), np.asarray(bd1, f)
    Wd2, bd2 = np.asarray(Wd2, f), np.asarray(bd2, f)

    enc0T = np.maximum(x @ We.T + be, 0.0).T          # [H, B]
    M = We @ Wd2                                       # [H, H]
    bM = We @ bd2 + be                                 # [H]

    wd1T = np.ascontiguousarray(Wd1.T).astype(bf)
    wmT = np.ascontiguousarray(M.T).astype(bf)
    wd2T = np.ascontiguousarray(Wd2.T).astype(bf)
    bd1c = np.ascontiguousarray(bd1[None, :]).astype(bf)
    bmc = np.ascontiguousarray(bM[None, :]).astype(bf)

    in_maps = []
    for k in range(NCORES):
        g, r = k // GP, k % GP
        rows = np.concatenate(
            [np.arange(q * H + r * SH, q * H + (r + 1) * SH) for q in GATE_ORDER]
        )
        m = {
            "wd1": wd1T, "wm": wmT, "wd2": wd2T, "bd1": bd1c, "bm": bmc,
            "enc0": np.ascontiguousarray(enc0T[:, g * BH:(g + 1) * BH]).astype(bf),
        }
        for l in range(L):
            m[f"wih{l}"] = np.ascontiguousarray(W_ih[l][rows, :].T).astype(bf)
            m[f"whh{l}"] = np.ascontiguousarray(W_hh[l][rows, :].T).astype(bf)
            bsum = (b_ih[l] + b_hh[l])[rows]
            m[f"bg{l}"] = np.ascontiguousarray(bsum[None, :]).astype(bf)
        in_maps.append(m)
    return in_maps, bd2


def kernel(x, We, be, W_ih, W_hh, b_ih, b_hh, Wd1, bd1, Wd2, bd2, timesteps, **run_kw):
    tsteps = int(timesteps)
    nc = _get_program(tsteps)
    in_maps, bd2_np = _prep_inputs(x, We, be, W_ih, W_hh, b_ih, b_hh, Wd1, bd1, Wd2, bd2)
    res = run_bass_kernel_spmd(nc, in_maps, core_ids=list(range(NCORES)), **run_kw)
    kernel.last_results = res
    halves = [np.asarray(res.results[g * GP]["out"], np.float32) for g in range(NG)]
    out = np.concatenate(halves, axis=0) + bd2_np[None, None, :]
    return out


# revision 10
# speedup vs baseline: 1.7018x; 1.0790x over previous
"""Trainium2 Bass kernel for the DigitalTwinModel (3-layer LSTM digital twin).

Strategy: hybrid MP-4 x DP-2 in bf16.
  - The 8 cores form two replica groups {0..3} and {4..7}; each group owns a
    batch half (128 rows).  Within a group the hidden dim is sharded 4-way:
    core (g, r) owns hidden features r*256:(r+1)*256 of every LSTM layer's
    h/c state plus the matching 4x256 gate rows of W_ih/W_hh.
  - Everything is bf16 on the wire and in the PE (fp32 PSUM accumulate,
    fp32 elementwise/cell state): a 1/4 weight shard fits SBUF-resident,
    AllGather payloads halve ([1024,128] bf16 out = 256KB -> ~21.5us), and
    the PE runs at full rate at batch=128 free size.
  - 3 AllGathers per timestep (one per layer, 4-rank groups).  The decoder
    (Wd1+relu, then M = We@Wd2 which fuses the output projection with the
    re-encode) is replicated per core; out[:,t,:] is produced batch-major
    via matmul(lhsT=rT, rhs=Wd2^T) one step deferred inside the next AG0
    window, together with the W_hh prefetches into the gate PSUMs.
  - Gate PSUM is split per 2KB bank (pgXa = g,g,i,i / pgXb = f,f,o,o) so
    the elementwise chain can start as soon as the first bank's
    accumulation group closes instead of waiting for all 64 matmuls.
    Decoder PSUMs reuse the gate banks in dead lifetime windows.
  - Gather buffers use a [P, rank*(2B)] layout so DMA runs are 512B and
    dodge the sub-512B descriptor penalty.
  - Tunable filler matmuls on resident weights keep the PE p-state ramped
    through the collective windows so every critical-path matmul burst
    runs at full clock.
"""

import numpy as np
import ml_dtypes

import concourse.bass as bass
import concourse.mybir as mybir
from concourse import bacc
import concourse.tile as tile
from concourse.bass_utils import run_bass_kernel_spmd

F32 = mybir.dt.float32
BF16 = mybir.dt.bfloat16
AF = mybir.ActivationFunctionType

B, D_IN, H, L, T = 256, 512, 1024, 3, 32
NCORES = 8
GP = 4                     # ranks per replica group
NG = NCORES // GP          # replica groups (data-parallel)
BH = B // NG               # batch rows per group
P = 128
SH = H // GP               # hidden features owned per core (256)
KT_H = H // P              # 8 k-tiles over the hidden dim
MT_G = 4 * SH // P         # 8 m-tiles of gates per core
HB = MT_G // 2             # 4 m-tiles per PSUM bank
GROUPS = [[0, 1, 2, 3], [4, 5, 6, 7]]
# gate m-tile order: (g,g,i,i | f,f,o,o) -> bank A holds g,i; bank B f,o.
GATE_ORDER = [2, 0, 1, 3]  # torch gate chunks: i=0, f=1, g=2, o=3

# PE p-state filler matmuls per collective window (each ~213ns at 2.4GHz)
FILL = (100, 114, 140)


def _touch(nc, ap2d):
    """Tiny ldweights that makes the PE observe a tile's producer semaphore
    (fused matmuls have a single sync-wait slot)."""
    nc.tensor.ldweights(weights=ap2d[0:1, 0:2].bitcast(BF16))


def build_program(timesteps=T):
    nc = bacc.Bacc(None, num_devices=NCORES, dynamic_dma_scratch_size=2048)

    # ---- kernel I/O (per-core payloads supplied from the host) ----
    wih = [nc.dram_tensor(f"wih{l}", [H, 4 * SH], BF16, kind="ExternalInput") for l in range(L)]
    whh = [nc.dram_tensor(f"whh{l}", [H, 4 * SH], BF16, kind="ExternalInput") for l in range(L)]
    bg = [nc.dram_tensor(f"bg{l}", [1, 4 * SH], BF16, kind="ExternalInput") for l in range(L)]
    wd1 = nc.dram_tensor("wd1", [H, H], BF16, kind="ExternalInput")
    wm = nc.dram_tensor("wm", [H, H], BF16, kind="ExternalInput")
    wd2 = nc.dram_tensor("wd2", [H, D_IN], BF16, kind="ExternalInput")
    bd1 = nc.dram_tensor("bd1", [1, H], BF16, kind="ExternalInput")
    bm = nc.dram_tensor("bm", [1, H], BF16, kind="ExternalInput")
    enc0 = nc.dram_tensor("enc0", [H, BH], BF16, kind="ExternalInput")
    out = nc.dram_tensor("out", [BH, timesteps, D_IN], F32, kind="ExternalOutput")

    with tile.TileContext(nc) as tc:
        with (
            tc.tile_pool(name="singles", bufs=1) as singles,
            tc.tile_pool(name="acts", bufs=1) as acts,
            tc.tile_pool(name="gtmp", bufs=1) as gtmp,
            tc.tile_pool(name="hloc", bufs=2) as hlocp,
            tc.tile_pool(name="obuf", bufs=1) as obuf,
            tc.tile_pool(name="pg", bufs=1, space="PSUM") as pgp,
            tc.tile_pool(name="dram", bufs=2, space="DRAM") as dram,
        ):
            # ---- load resident weights/biases into SBUF ----
            s_wih, s_whh, s_bg = [], [], []
            for l in range(L):
                w = singles.tile([P, KT_H, 4 * SH], BF16, tag=f"swih{l}", name=f"swih{l}")
                nc.sync.dma_start(out=w, in_=wih[l][:].rearrange("(kk p) m -> p kk m", p=P))
                _touch(nc, w[:, 0, :])
                s_wih.append(w)
            for l in range(L):
                w = singles.tile([P, KT_H, 4 * SH], BF16, tag=f"swhh{l}", name=f"swhh{l}")
                nc.sync.dma_start(out=w, in_=whh[l][:].rearrange("(kk p) m -> p kk m", p=P))
                _touch(nc, w[:, 0, :])
                s_whh.append(w)
            for l in range(L):
                t_ = singles.tile([1, 4 * SH], BF16, tag=f"sbg{l}", name=f"sbg{l}")
                nc.sync.dma_start(out=t_, in_=bg[l][:])
                s_bg.append(t_)
            s_wd1 = singles.tile([P, KT_H, H], BF16, tag="swd1", name="swd1")
            nc.sync.dma_start(out=s_wd1, in_=wd1[:].rearrange("(kk p) m -> p kk m", p=P))
            _touch(nc, s_wd1[:, 0, :])
            s_wm = singles.tile([P, KT_H, H], BF16, tag="swm", name="swm")
            nc.sync.dma_start(out=s_wm, in_=wm[:].rearrange("(kk p) m -> p kk m", p=P))
            _touch(nc, s_wm[:, 0, :])
            s_wd2 = singles.tile([P, KT_H, D_IN], BF16, tag="swd2", name="swd2")
            nc.sync.dma_start(out=s_wd2, in_=wd2[:].rearrange("(kk p) m -> p kk m", p=P))
            _touch(nc, s_wd2[:, 0, :])
            s_bd1 = singles.tile([1, H], BF16, tag="sbd1", name="sbd1")
            nc.sync.dma_start(out=s_bd1, in_=bd1[:])
            s_bm = singles.tile([1, H], BF16, tag="sbm", name="sbm")
            nc.sync.dma_start(out=s_bm, in_=bm[:])
            ones = singles.tile([1, BH], BF16, tag="ones", name="ones")
            nc.vector.memset(ones, 1.0)

            s_c = []
            for l in range(L):
                c = singles.tile([P, 2, BH], F32, tag=f"c{l}", name=f"c{l}")
                nc.vector.memset(c, 0.0)
                s_c.append(c)

            def pghalf(tag, name):
                return pgp.tile([P, HB, BH], F32, tag=tag, name=name)

            def bias_fold(pa, pb, btile):
                """Open both banks' accumulation groups; add per-(partition,
                m-tile) biases via rank-1 ones matmuls. start=True on the
                bank-first m-tile clears the whole 2KiB bank."""
                for half, pt in ((0, pa), (1, pb)):
                    for m in range(HB):
                        nc.tensor.matmul(
                            pt[:, m, :],
                            lhsT=btile[0:1, (half * HB + m) * P:(half * HB + m + 1) * P],
                            rhs=ones[0:1, :],
                            start=(m == 0),
                            stop=False,
                        )

            def rhs_kt(hT, kk):
                """k-tile kk of a gathered tensor in [P, GP, 2*BH]... note:
                hT here is [P, GP, 2, BH]; kk maps to (rank, half)."""
                return hT[:, kk // 2, kk % 2, :]

            def mm_gates(pa, pb, w, hT, close):
                """Accumulate w^T @ hT into the two bank tiles, bank-A m-tiles
                first so elementwise can start while bank B accumulates."""
                for half, pt in ((0, pa), (1, pb)):
                    for m in range(HB):
                        for kk in range(KT_H):
                            nc.tensor.matmul(
                                pt[:, m, :],
                                lhsT=w[:, kk, (half * HB + m) * P:(half * HB + m + 1) * P],
                                rhs=rhs_kt(hT, kk),
                                start=False,
                                stop=(close and kk == KT_H - 1 and m == HB - 1),
                            )

            def mm_dense(pa, pb, w, xa, xb, close, kk_outer=False):
                """Like mm_gates but rhs is a local tensor split into two
                [P, HB, BH] half tiles. kk_outer=True consumes the halves
                incrementally (for chains where they become ready in order)."""
                def xkt(kk):
                    return (xa if kk < HB else xb)[:, kk % HB, :]
                if kk_outer:
                    for phase in range(2):
                        for kk in range(phase * HB, (phase + 1) * HB):
                            for half, pt in ((0, pa), (1, pb)):
                                for m in range(HB):
                                    nc.tensor.matmul(
                                        pt[:, m, :],
                                        lhsT=w[:, kk, (half * HB + m) * P:(half * HB + m + 1) * P],
                                        rhs=xkt(kk),
                                        start=False,
                                        stop=(close and kk == KT_H - 1 and m == HB - 1),
                                    )
                else:
                    for half, pt in ((0, pa), (1, pb)):
                        for m in range(HB):
                            for kk in range(KT_H):
                                nc.tensor.matmul(
                                    pt[:, m, :],
                                    lhsT=w[:, kk, (half * HB + m) * P:(half * HB + m + 1) * P],
                                    rhs=xkt(kk),
                                    start=False,
                                    stop=(close and kk == KT_H - 1 and m == HB - 1),
                                )

            def elementwise(l, pa, pb):
                """bank A = (g,g,i,i), bank B = (f,f,o,o) -> h'_l (bf16)."""
                tg = gtmp.tile([P, 2, BH], F32, tag="tg", name="tg")
                si = gtmp.tile([P, 2, BH], F32, tag="si", name="si")
                sfo = gtmp.tile([P, 4, BH], F32, tag="sfo", name="sfo")
                t1 = gtmp.tile([P, 2, BH], F32, tag="t1", name="t1")
                t2 = gtmp.tile([P, 2, BH], F32, tag="t2", name="t2")
                tc_ = gtmp.tile([P, 2, BH], F32, tag="tc", name="tc")
                nc.scalar.activation(tg, pa[:, 0:2, :], AF.Tanh)
                nc.scalar.activation(si, pa[:, 2:4, :], AF.Sigmoid)
                nc.vector.tensor_mul(t1, si, tg)                   # i * g
                nc.scalar.activation(sfo, pb, AF.Sigmoid)
                nc.vector.tensor_mul(t2, sfo[:, 0:2, :], s_c[l])   # f * c
                nc.vector.tensor_add(s_c[l], t1, t2)
                nc.scalar.activation(tc_, s_c[l], AF.Tanh)
                hl = hlocp.tile([P, 2, BH], BF16, tag=f"hl{l}", name=f"hl{l}")
                nc.vector.tensor_mul(hl, sfo[:, 2:4, :], tc_)
                return hl

            def allgather(hl, l):
                # agin row p = [c0 batch..., c1 batch...]: 512B contiguous
                agin = dram.tile([P, SH // P * BH], BF16, tag=f"agin{l}", name=f"agin{l}")
                agout = dram.tile([GP * P, SH // P * BH], BF16, tag=f"agout{l}", name=f"agout{l}")
                nc.sync.dma_start(out=agin, in_=hl[:].rearrange("p c b -> p (c b)"))
                nc.gpsimd.collective_compute(
                    "AllGather",
                    mybir.AluOpType.bypass,
                    replica_groups=GROUPS,
                    ins=[agin.opt()],
                    outs=[agout.opt()],
                )
                return agout

            def fetch_hT(agout, l):
                # [P, rank, half, BH]; in-side rows are 512B contiguous
                hT = acts.tile([P, GP, 2, BH], BF16, tag=f"hT{l}", name=f"hT{l}")
                nc.sync.dma_start(
                    out=hT[:].rearrange("p r c b -> p r (c b)"),
                    in_=agout[:].rearrange("(r p) x -> p r x", p=P))
                return hT

            def emit_outbt(rTa, rTb, tstep):
                """out[:, tstep, :] = (rT^T @ Wd2^T); bd2 added on host."""
                _touch(nc, rTa[:, 0, :])
                _touch(nc, rTb[:, 0, :])
                po = pghalf("pg0a", "po")
                pov = po[:].rearrange("p a b -> p (a b)")
                for kk in range(KT_H):
                    nc.tensor.matmul(
                        pov,
                        lhsT=(rTa if kk < HB else rTb)[:, kk % HB, :],
                        rhs=s_wd2[:, kk, :],
                        start=kk == 0,
                        stop=kk == KT_H - 1,
                    )
                ob = obuf.tile([P, D_IN], F32, tag="ob", name="ob")
                nc.vector.tensor_copy(out=ob, in_=pov)
                nc.sync.dma_start(out=out[:, tstep, :], in_=ob)

            # ---- PE p-state filler ----
            pfill = pgp.tile([P, D_IN], F32, tag="fill", name="pfill")

            def fill(n):
                for _ in range(n):
                    nc.tensor.matmul(
                        pfill,
                        lhsT=s_wd1[:, 0, 0:P],
                        rhs=s_wd1[:, 1, 0:D_IN],
                        start=True,
                        stop=True,
                        skip_group_check=True,
                    )

            # ---- prologue: enc(0); open gate groups (h(-1) = 0) ----
            enca = acts.tile([P, HB, BH], BF16, tag="enca", name="enca")
            encb = acts.tile([P, HB, BH], BF16, tag="encb", name="encb")
            nc.sync.dma_start(
                out=enca, in_=enc0[0:HB * P, :].rearrange("(kk p) b -> p kk b", p=P))
            nc.sync.dma_start(
                out=encb, in_=enc0[HB * P:H, :].rearrange("(kk p) b -> p kk b", p=P))
            pga = [None] * L
            pgb = [None] * L
            for l in range(L):
                pga[l] = pghalf(f"pg{l}a", f"pg{l}a")
                pgb[l] = pghalf(f"pg{l}b", f"pg{l}b")
                bias_fold(pga[l], pgb[l], s_bg[l])

            hT = [None] * L
            rT_prev, t_prev = None, None

            for t in range(timesteps):
                # ---- layer 0 gates (enc from prologue / dec chain) ----
                mm_dense(pga[0], pgb[0], s_wih[0], enca, encb, close=True,
                         kk_outer=(t > 0))
                h0l = elementwise(0, pga[0], pgb[0])
                ag0 = allgather(h0l, 0)
                # AG0 window: deferred out-write, W_hh1/W_hh2 prefetch
                if t > 0:
                    emit_outbt(rT_prev[0], rT_prev[1], t_prev)
                    _touch(nc, hT[1][:, 0, 0, :])
                    pga[1], pgb[1] = pghalf("pg1a", "pg1a"), pghalf("pg1b", "pg1b")
                    bias_fold(pga[1], pgb[1], s_bg[1])
                    mm_gates(pga[1], pgb[1], s_whh[1], hT[1], close=False)
                    _touch(nc, hT[2][:, 0, 0, :])
                    pga[2], pgb[2] = pghalf("pg2a", "pg2a"), pghalf("pg2b", "pg2b")
                    bias_fold(pga[2], pgb[2], s_bg[2])
                    mm_gates(pga[2], pgb[2], s_whh[2], hT[2], close=False)
                fill(FILL[0])
                hT[0] = fetch_hT(ag0, 0)

                # ---- layer 1 ----
                _touch(nc, hT[0][:, 0, 0, :])
                mm_gates(pga[1], pgb[1], s_wih[1], hT[0], close=True)
                h1l = elementwise(1, pga[1], pgb[1])
                ag1 = allgather(h1l, 1)
                # AG1 window: W_hh0 prefetch for t+1
                if t + 1 < timesteps:
                    pga[0], pgb[0] = pghalf("pg0a", "pg0a"), pghalf("pg0b", "pg0b")
                    bias_fold(pga[0], pgb[0], s_bg[0])
                    mm_gates(pga[0], pgb[0], s_whh[0], hT[0], close=False)
                fill(FILL[1])
                hT[1] = fetch_hT(ag1, 1)

                # ---- layer 2 ----
                _touch(nc, hT[1][:, 0, 0, :])
                mm_gates(pga[2], pgb[2], s_wih[2], hT[1], close=True)
                h2l = elementwise(2, pga[2], pgb[2])
                ag2 = allgather(h2l, 2)
                fill(FILL[2])
                hT[2] = fetch_hT(ag2, 2)

                # ---- decoder chain: d1 -> rT -> (M -> enc -> G0) ----
                _touch(nc, hT[2][:, 0, 0, :])
                pd1a, pd1b = pghalf("pg1a", "pd1a"), pghalf("pg1b", "pd1b")
                bias_fold(pd1a, pd1b, s_bd1)
                mm_gates(pd1a, pd1b, s_wd1, hT[2], close=True)
                rTa = acts.tile([P, HB, BH], BF16, tag="rTa", name="rTa")
                rTb = acts.tile([P, HB, BH], BF16, tag="rTb", name="rTb")
                nc.scalar.activation(rTa, pd1a, AF.Relu)
                nc.scalar.activation(rTb, pd1b, AF.Relu)

                if t + 1 < timesteps:
                    _touch(nc, rTa[:, 0, :])
                    pea, peb = pghalf("pg2a", "pea"), pghalf("pg2b", "peb")
                    bias_fold(pea, peb, s_bm)
                    _touch(nc, rTb[:, 0, :])
                    mm_dense(pea, peb, s_wm, rTa, rTb, close=True, kk_outer=True)
                    enca = acts.tile([P, HB, BH], BF16, tag="enca", name="enca")
                    encb = acts.tile([P, HB, BH], BF16, tag="encb", name="encb")
                    nc.scalar.activation(enca, pea, AF.Relu)
                    nc.scalar.activation(encb, peb, AF.Relu)
                    rT_prev, t_prev = (rTa, rTb), t
                else:
                    emit_outbt(rTa, rTb, t)

    nc.compile()
    return nc


_CACHE = {}


def _get_program(timesteps):
    if timesteps not in _CACHE:
        _CACHE[timesteps] = build_program(timesteps)
    return _CACHE[timesteps]


def _prep_inputs(x, We, be, W_ih, W_hh, b_ih, b_hh, Wd1, bd1, Wd2, bd2):
    """Host-side layout: bf16 weights, per-core gate-row shards, folded
    decoder matrix M = We@Wd2, batch halves per replica group."""
    f = np.float32
    bf = ml_dtypes.bfloat16
    x, We, be = np.asarray(x, f), np.asarray(We, f), np.asarray(be, f)
    W_ih, W_hh = np.asarray(W_ih, f), np.asarray(W_hh, f)
    b_ih, b_hh = np.asarray(b_ih, f), np.asarray(b_hh, f)
    Wd1, bd1 = np.asarray(Wd1, f), np.asarray(bd1, f)
    Wd2, bd2 = np.asarray(Wd2, f), np.asarray(bd2, f)

    enc0T = np.maximum(x @ We.T + be, 0.0).T          # [H, B]
    M = We @ Wd2                                       # [H, H]
    bM = We @ bd2 + be                                 # [H]

    wd1T = np.ascontiguousarray(Wd1.T).astype(bf)
    wmT = np.ascontiguousarray(M.T).astype(bf)
    wd2T = np.ascontiguousarray(Wd2.T).astype(bf)
    bd1c = np.ascontiguousarray(bd1[None, :]).astype(bf)
    bmc = np.ascontiguousarray(bM[None, :]).astype(bf)

    in_maps = []
    for k in range(NCORES):
        g, r = k // GP, k % GP
        rows = np.concatenate(
            [np.arange(q * H + r * SH, q * H + (r + 1) * SH) for q in GATE_ORDER]
        )
        m = {
            "wd1": wd1T, "wm": wmT, "wd2": wd2T, "bd1": bd1c, "bm": bmc,
            "enc0": np.ascontiguousarray(enc0T[:, g * BH:(g + 1) * BH]).astype(bf),
        }
        for l in range(L):
            m[f"wih{l}"] = np.ascontiguousarray(W_ih[l][rows, :].T).astype(bf)
            m[f"whh{l}"] = np.ascontiguousarray(W_hh[l][rows, :].T).astype(bf)
            bsum = (b_ih[l] + b_hh[l])[rows]
            m[f"bg{l}"] = np.ascontiguousarray(bsum[None, :]).astype(bf)
        in_maps.append(m)
    return in_maps, bd2


def kernel(x, We, be, W_ih, W_hh, b_ih, b_hh, Wd1, bd1, Wd2, bd2, timesteps, **run_kw):
    tsteps = int(timesteps)
    nc = _get_program(tsteps)
    in_maps, bd2_np = _prep_inputs(x, We, be, W_ih, W_hh, b_ih, b_hh, Wd1, bd1, Wd2, bd2)
    res = run_bass_kernel_spmd(nc, in_maps, core_ids=list(range(NCORES)), **run_kw)
    kernel.last_results = res
    halves = [np.asarray(res.results[g * GP]["out"], np.float32) for g in range(NG)]
    out = np.concatenate(halves, axis=0) + bd2_np[None, None, :]
    return out


# revision 16
# speedup vs baseline: 1.7115x; 1.0057x over previous
"""Trainium2 Bass kernel for the DigitalTwinModel (3-layer LSTM digital twin).

Strategy: hybrid MP-4 x DP-2 in bf16.
  - The 8 cores form two replica groups {0..3} and {4..7}; each group owns a
    batch half (128 rows).  Within a group the hidden dim is sharded 4-way:
    core (g, r) owns hidden features r*256:(r+1)*256 of every LSTM layer's
    h/c state plus the matching 4x256 gate rows of W_ih/W_hh.
  - Everything is bf16 on the wire and in the PE (fp32 PSUM accumulate,
    fp32 elementwise/cell state): a 1/4 weight shard fits SBUF-resident,
    AllGather payloads halve ([1024,128] bf16 out = 256KB -> ~21.5us), and
    the PE runs at full rate at batch=128 free size.
  - 3 AllGathers per timestep (one per layer, 4-rank groups).  The decoder
    (Wd1+relu, then M = We@Wd2 which fuses the output projection with the
    re-encode) is replicated per core; out[:,t,:] is produced batch-major
    via matmul(lhsT=rT, rhs=Wd2^T) one step deferred inside the next AG0
    window, together with the W_hh prefetches into the gate PSUMs.
  - Gate PSUM is split per 2KB bank (pgXa = g,g,i,i / pgXb = f,f,o,o) so
    the elementwise chain can start as soon as the first bank's
    accumulation group closes instead of waiting for all 64 matmuls.
    Decoder PSUMs reuse the gate banks in dead lifetime windows.
  - Gather buffers use a [P, rank*(2B)] layout so DMA runs are 512B and
    dodge the sub-512B descriptor penalty.
  - Tunable filler matmuls on resident weights keep the PE p-state ramped
    through the collective windows so every critical-path matmul burst
    runs at full clock.
"""

import numpy as np
import ml_dtypes

import concourse.bass as bass
import concourse.mybir as mybir
from concourse import bacc
import concourse.tile as tile
from concourse.bass_utils import run_bass_kernel_spmd

F32 = mybir.dt.float32
BF16 = mybir.dt.bfloat16
AF = mybir.ActivationFunctionType

B, D_IN, H, L, T = 256, 512, 1024, 3, 32
NCORES = 8
GP = 4                     # ranks per replica group
NG = NCORES // GP          # replica groups (data-parallel)
BH = B // NG               # batch rows per group
P = 128
SH = H // GP               # hidden features owned per core (256)
KT_H = H // P              # 8 k-tiles over the hidden dim
MT_G = 4 * SH // P         # 8 m-tiles of gates per core
HB = MT_G // 2             # 4 m-tiles per PSUM bank
GROUPS = [[0, 1, 2, 3], [4, 5, 6, 7]]
# gate m-tile order: (g,g,i,i | f,f,o,o) -> bank A holds g,i; bank B f,o.
GATE_ORDER = [2, 0, 1, 3]  # torch gate chunks: i=0, f=1, g=2, o=3

# PE p-state filler matmuls per collective window (each ~213ns at 2.4GHz)
FILL = (100, 114, 140)


def _touch(nc, ap2d):
    """Tiny ldweights that makes the PE observe a tile's producer semaphore
    (fused matmuls have a single sync-wait slot)."""
    nc.tensor.ldweights(weights=ap2d[0:1, 0:2].bitcast(BF16))


def build_program(timesteps=T):
    nc = bacc.Bacc(None, num_devices=NCORES, dynamic_dma_scratch_size=2048)

    # ---- kernel I/O (per-core payloads supplied from the host) ----
    wih = [nc.dram_tensor(f"wih{l}", [H, 4 * SH], BF16, kind="ExternalInput") for l in range(L)]
    whh = [nc.dram_tensor(f"whh{l}", [H, 4 * SH], BF16, kind="ExternalInput") for l in range(L)]
    bg = [nc.dram_tensor(f"bg{l}", [1, 4 * SH], BF16, kind="ExternalInput") for l in range(L)]
    wd1 = nc.dram_tensor("wd1", [H, H], BF16, kind="ExternalInput")
    wm = nc.dram_tensor("wm", [H, H], BF16, kind="ExternalInput")
    wd2 = nc.dram_tensor("wd2", [H, D_IN], BF16, kind="ExternalInput")
    bd1 = nc.dram_tensor("bd1", [1, H], BF16, kind="ExternalInput")
    bm = nc.dram_tensor("bm", [1, H], BF16, kind="ExternalInput")
    enc0 = nc.dram_tensor("enc0", [H, BH], BF16, kind="ExternalInput")
    out = nc.dram_tensor("out", [BH, timesteps, D_IN], F32, kind="ExternalOutput")

    with tile.TileContext(nc) as tc:
        with (
            tc.tile_pool(name="singles", bufs=1) as singles,
            tc.tile_pool(name="acts", bufs=1) as acts,
            tc.tile_pool(name="gtmp", bufs=1) as gtmp,
            tc.tile_pool(name="hloc", bufs=2) as hlocp,
            tc.tile_pool(name="obuf", bufs=1) as obuf,
            tc.tile_pool(name="pg", bufs=1, space="PSUM") as pgp,
            tc.tile_pool(name="dram", bufs=2, space="DRAM") as dram,
        ):
            # ---- load resident weights/biases into SBUF ----
            # Emission order follows first use at t=0 so the first step's
            # chain isn't queued behind 18MB of weight DMAs: enc0 + layer-0
            # gates first, decoder weights next, W_hh (only needed from the
            # first prefetch windows) and Wd2 (deferred out-write) last.
            ones = singles.tile([1, BH], BF16, tag="ones", name="ones")
            nc.vector.memset(ones, 1.0)
            s_c = []
            for l in range(L):
                c = singles.tile([P, 2, BH], F32, tag=f"c{l}", name=f"c{l}")
                nc.vector.memset(c, 0.0)
                s_c.append(c)

            s_bg = []
            for l in range(L):
                t_ = singles.tile([1, 4 * SH], BF16, tag=f"sbg{l}", name=f"sbg{l}")
                nc.sync.dma_start(out=t_, in_=bg[l][:])
                s_bg.append(t_)
            s_bd1 = singles.tile([1, H], BF16, tag="sbd1", name="sbd1")
            nc.sync.dma_start(out=s_bd1, in_=bd1[:])
            s_bm = singles.tile([1, H], BF16, tag="sbm", name="sbm")
            nc.sync.dma_start(out=s_bm, in_=bm[:])

            # (first-use _touch calls are emitted inside the loop at t=0/1 so
            # the PE isn't head-of-line blocked on late weight DMAs)
            s_wih, s_whh = [], []
            for l in range(L):
                w = singles.tile([P, KT_H, 4 * SH], BF16, tag=f"swih{l}", name=f"swih{l}")
                nc.sync.dma_start(out=w, in_=wih[l][:].rearrange("(kk p) m -> p kk m", p=P))
                if l == 0:
                    _touch(nc, w[:, 0, :])
                s_wih.append(w)
            s_wd1 = singles.tile([P, KT_H, H], BF16, tag="swd1", name="swd1")
            nc.sync.dma_start(out=s_wd1, in_=wd1[:].rearrange("(kk p) m -> p kk m", p=P))
            s_wm = singles.tile([P, KT_H, H], BF16, tag="swm", name="swm")
            nc.sync.dma_start(out=s_wm, in_=wm[:].rearrange("(kk p) m -> p kk m", p=P))
            for l in range(L):
                w = singles.tile([P, KT_H, 4 * SH], BF16, tag=f"swhh{l}", name=f"swhh{l}")
                nc.sync.dma_start(out=w, in_=whh[l][:].rearrange("(kk p) m -> p kk m", p=P))
                s_whh.append(w)
            s_wd2 = singles.tile([P, KT_H, D_IN], BF16, tag="swd2", name="swd2")
            nc.sync.dma_start(out=s_wd2, in_=wd2[:].rearrange("(kk p) m -> p kk m", p=P))

            def pghalf(tag, name):
                return pgp.tile([P, HB, BH], F32, tag=tag, name=name)

            def bias_fold(pa, pb, btile):
                """Open both banks' accumulation groups; add per-(partition,
                m-tile) biases via rank-1 ones matmuls. start=True on the
                bank-first m-tile clears the whole 2KiB bank."""
                for half, pt in ((0, pa), (1, pb)):
                    for m in range(HB):
                        nc.tensor.matmul(
                            pt[:, m, :],
                            lhsT=btile[0:1, (half * HB + m) * P:(half * HB + m + 1) * P],
                            rhs=ones[0:1, :],
                            start=(m == 0),
                            stop=False,
                        )

            def rhs_kt(hT, kk):
                """k-tile kk of a gathered tensor in [P, GP, 2*BH]... note:
                hT here is [P, GP, 2, BH]; kk maps to (rank, half)."""
                return hT[:, kk // 2, kk % 2, :]

            def mm_gates(pa, pb, w, hT, close):
                """Accumulate w^T @ hT into the two bank tiles, bank-A m-tiles
                first so elementwise can start while bank B accumulates."""
                for half, pt in ((0, pa), (1, pb)):
                    for m in range(HB):
                        for kk in range(KT_H):
                            nc.tensor.matmul(
                                pt[:, m, :],
                                lhsT=w[:, kk, (half * HB + m) * P:(half * HB + m + 1) * P],
                                rhs=rhs_kt(hT, kk),
                                start=False,
                                stop=(close and kk == KT_H - 1 and m == HB - 1),
                            )

            def mm_dense(pa, pb, w, xa, xb, close, kk_outer=False):
                """Like mm_gates but rhs is a local tensor split into two
                [P, HB, BH] half tiles. kk_outer=True consumes the halves
                incrementally (for chains where they become ready in order)."""
                def xkt(kk):
                    return (xa if kk < HB else xb)[:, kk % HB, :]
                if kk_outer:
                    for phase in range(2):
                        for kk in range(phase * HB, (phase + 1) * HB):
                            for half, pt in ((0, pa), (1, pb)):
                                for m in range(HB):
                                    nc.tensor.matmul(
                                        pt[:, m, :],
                                        lhsT=w[:, kk, (half * HB + m) * P:(half * HB + m + 1) * P],
                                        rhs=xkt(kk),
                                        start=False,
                                        stop=(close and kk == KT_H - 1 and m == HB - 1),
                                    )
                else:
                    for half, pt in ((0, pa), (1, pb)):
                        for m in range(HB):
                            for kk in range(KT_H):
                                nc.tensor.matmul(
                                    pt[:, m, :],
                                    lhsT=w[:, kk, (half * HB + m) * P:(half * HB + m + 1) * P],
                                    rhs=xkt(kk),
                                    start=False,
                                    stop=(close and kk == KT_H - 1 and m == HB - 1),
                                )

            def elementwise(l, pa, pb):
                """bank A = (g,g,i,i), bank B = (f,f,o,o) -> h'_l (bf16)."""
                tg = gtmp.tile([P, 2, BH], F32, tag="tg", name="tg")
                si = gtmp.tile([P, 2, BH], F32, tag="si", name="si")
                sf = gtmp.tile([P, 2, BH], F32, tag="sf", name="sf")
                so = gtmp.tile([P, 2, BH], F32, tag="so", name="so")
                t1 = gtmp.tile([P, 2, BH], F32, tag="t1", name="t1")
                t2 = gtmp.tile([P, 2, BH], F32, tag="t2", name="t2")
                tc_ = gtmp.tile([P, 2, BH], F32, tag="tc", name="tc")
                nc.scalar.activation(tg, pa[:, 0:2, :], AF.Tanh)
                nc.scalar.activation(si, pa[:, 2:4, :], AF.Sigmoid)
                nc.vector.tensor_mul(t1, si, tg)                   # i * g
                nc.scalar.activation(sf, pb[:, 0:2, :], AF.Sigmoid)
                nc.vector.tensor_mul(t2, sf, s_c[l])               # f * c
                nc.scalar.activation(so, pb[:, 2:4, :], AF.Sigmoid)
                nc.vector.tensor_add(s_c[l], t1, t2)
                nc.scalar.activation(tc_, s_c[l], AF.Tanh)
                hl = hlocp.tile([P, 2, BH], BF16, tag=f"hl{l}", name=f"hl{l}")
                nc.vector.tensor_mul(hl, so, tc_)
                return hl

            def allgather(hl, l):
                # agin row p = [c0 batch..., c1 batch...]: 512B contiguous
                agin = dram.tile([P, SH // P * BH], BF16, tag=f"agin{l}", name=f"agin{l}")
                agout = dram.tile([GP * P, SH // P * BH], BF16, tag=f"agout{l}", name=f"agout{l}")
                nc.sync.dma_start(out=agin, in_=hl[:].rearrange("p c b -> p (c b)"))
                nc.gpsimd.collective_compute(
                    "AllGather",
                    mybir.AluOpType.bypass,
                    replica_groups=GROUPS,
                    ins=[agin.opt()],
                    outs=[agout.opt()],
                )
                return agout

            def fetch_hT(agout, l):
                # [P, rank, half, BH]; in-side rows are 512B contiguous
                hT = acts.tile([P, GP, 2, BH], BF16, tag=f"hT{l}", name=f"hT{l}")
                nc.sync.dma_start(
                    out=hT[:].rearrange("p r c b -> p r (c b)"),
                    in_=agout[:].rearrange("(r p) x -> p r x", p=P))
                return hT

            def emit_outbt(rTa, rTb, tstep):
                """out[:, tstep, :] = (rT^T @ Wd2^T); bd2 added on host."""
                _touch(nc, rTa[:, 0, :])
                _touch(nc, rTb[:, 0, :])
                po = pghalf("pg0a", "po")
                pov = po[:].rearrange("p a b -> p (a b)")
                for kk in range(KT_H):
                    nc.tensor.matmul(
                        pov,
                        lhsT=(rTa if kk < HB else rTb)[:, kk % HB, :],
                        rhs=s_wd2[:, kk, :],
                        start=kk == 0,
                        stop=kk == KT_H - 1,
                    )
                ob = obuf.tile([P, D_IN], F32, tag="ob", name="ob")
                nc.vector.tensor_copy(out=ob, in_=pov)
                nc.sync.dma_start(out=out[:, tstep, :], in_=ob)

            # ---- PE p-state filler ----
            pfill = pgp.tile([P, D_IN], F32, tag="fill", name="pfill")

            def fill(n):
                for _ in range(n):
                    nc.tensor.matmul(
                        pfill,
                        lhsT=s_wd1[:, 0, 0:P],
                        rhs=s_wd1[:, 1, 0:D_IN],
                        start=True,
                        stop=True,
                        skip_group_check=True,
                    )

            # ---- prologue: enc(0); open gate groups (h(-1) = 0) ----
            enca = acts.tile([P, HB, BH], BF16, tag="enca", name="enca")
            encb = acts.tile([P, HB, BH], BF16, tag="encb", name="encb")
            nc.sync.dma_start(
                out=enca, in_=enc0[0:HB * P, :].rearrange("(kk p) b -> p kk b", p=P))
            nc.sync.dma_start(
                out=encb, in_=enc0[HB * P:H, :].rearrange("(kk p) b -> p kk b", p=P))
            pga = [None] * L
            pgb = [None] * L
            for l in range(L):
                pga[l] = pghalf(f"pg{l}a", f"pg{l}a")
                pgb[l] = pghalf(f"pg{l}b", f"pg{l}b")
                bias_fold(pga[l], pgb[l], s_bg[l])

            hT = [None] * L
            rT_prev, t_prev = None, None

            for t in range(timesteps):
                # ---- layer 0 gates (enc from prologue / dec chain) ----
                mm_dense(pga[0], pgb[0], s_wih[0], enca, encb, close=True,
                         kk_outer=(t > 0))
                h0l = elementwise(0, pga[0], pgb[0])
                ag0 = allgather(h0l, 0)
                # AG0 window: deferred out-write, W_hh1/W_hh2 prefetch
                if t > 0:
                    emit_outbt(rT_prev[0], rT_prev[1], t_prev)
                    _touch(nc, hT[1][:, 0, 0, :])
                    pga[1], pgb[1] = pghalf("pg1a", "pg1a"), pghalf("pg1b", "pg1b")
                    bias_fold(pga[1], pgb[1], s_bg[1])
                    mm_gates(pga[1], pgb[1], s_whh[1], hT[1], close=False)
                    _touch(nc, hT[2][:, 0, 0, :])
                    pga[2], pgb[2] = pghalf("pg2a", "pg2a"), pghalf("pg2b", "pg2b")
                    bias_fold(pga[2], pgb[2], s_bg[2])
                    mm_gates(pga[2], pgb[2], s_whh[2], hT[2], close=False)
                fill(FILL[0])
                hT[0] = fetch_hT(ag0, 0)

                # ---- layer 1 ----
                _touch(nc, hT[0][:, 0, 0, :])
                mm_gates(pga[1], pgb[1], s_wih[1], hT[0], close=True)
                h1l = elementwise(1, pga[1], pgb[1])
                ag1 = allgather(h1l, 1)
                # AG1 window: W_hh0 prefetch for t+1
                if t + 1 < timesteps:
                    pga[0], pgb[0] = pghalf("pg0a", "pg0a"), pghalf("pg0b", "pg0b")
                    bias_fold(pga[0], pgb[0], s_bg[0])
                    mm_gates(pga[0], pgb[0], s_whh[0], hT[0], close=False)
                fill(FILL[1])
                hT[1] = fetch_hT(ag1, 1)

                # ---- layer 2 ----
                _touch(nc, hT[1][:, 0, 0, :])
                mm_gates(pga[2], pgb[2], s_wih[2], hT[1], close=True)
                h2l = elementwise(2, pga[2], pgb[2])
                ag2 = allgather(h2l, 2)
                fill(FILL[2])
                hT[2] = fetch_hT(ag2, 2)

                # ---- decoder chain: d1 -> rT -> (M -> enc -> G0) ----
                _touch(nc, hT[2][:, 0, 0, :])
                pd1a, pd1b = pghalf("pg1a", "pd1a"), pghalf("pg1b", "pd1b")
                bias_fold(pd1a, pd1b, s_bd1)
                mm_gates(pd1a, pd1b, s_wd1, hT[2], close=True)
                rTa = acts.tile([P, HB, BH], BF16, tag="rTa", name="rTa")
                rTb = acts.tile([P, HB, BH], BF16, tag="rTb", name="rTb")
                nc.scalar.activation(rTa, pd1a, AF.Relu)
                nc.scalar.activation(rTb, pd1b, AF.Relu)

                if t + 1 < timesteps:
                    _touch(nc, rTa[:, 0, :])
                    pea, peb = pghalf("pg2a", "pea"), pghalf("pg2b", "peb")
                    bias_fold(pea, peb, s_bm)
                    _touch(nc, rTb[:, 0, :])
                    mm_dense(pea, peb, s_wm, rTa, rTb, close=True, kk_outer=True)
                    enca = acts.tile([P, HB, BH], BF16, tag="enca", name="enca")
                    encb = acts.tile([P, HB, BH], BF16, tag="encb", name="encb")
                    nc.scalar.activation(enca, pea, AF.Relu)
                    nc.scalar.activation(encb, peb, AF.Relu)
                    rT_prev, t_prev = (rTa, rTb), t
                else:
                    emit_outbt(rTa, rTb, t)

    nc.compile()
    return nc


_CACHE = {}


def _get_program(timesteps):
    if timesteps not in _CACHE:
        _CACHE[timesteps] = build_program(timesteps)
    return _CACHE[timesteps]


def _prep_inputs(x, We, be, W_ih, W_hh, b_ih, b_hh, Wd1, bd1, Wd2, bd2):
    """Host-side layout: bf16 weights, per-core gate-row shards, folded
    decoder matrix M = We@Wd2, batch halves per replica group."""
    f = np.float32
    bf = ml_dtypes.bfloat16
    x, We, be = np.asarray(x, f), np.asarray(We, f), np.asarray(be, f)
    W_ih, W_hh = np.asarray(W_ih, f), np.asarray(W_hh, f)
    b_ih, b_hh = np.asarray(b_ih, f), np.asarray(b_hh, f)
    Wd1, bd1 = np.asarray(Wd1, f), np.asarray(bd1, f)
    Wd2, bd2 = np.asarray(Wd2, f), np.asarray(bd2, f)

    enc0T = np.maximum(x @ We.T + be, 0.0).T          # [H, B]
    M = We @ Wd2                                       # [H, H]
    bM = We @ bd2 + be                                 # [H]

    wd1T = np.ascontiguousarray(Wd1.T).astype(bf)
    wmT = np.ascontiguousarray(M.T).astype(bf)
    wd2T = np.ascontiguousarray(Wd2.T).astype(bf)
    bd1c = np.ascontiguousarray(bd1[None, :]).astype(bf)
    bmc = np.ascontiguousarray(bM[None, :]).astype(bf)

    in_maps = []
    for k in range(NCORES):
        g, r = k // GP, k % GP
        rows = np.concatenate(
            [np.arange(q * H + r * SH, q * H + (r + 1) * SH) for q in GATE_ORDER]
        )
        m = {
            "wd1": wd1T, "wm": wmT, "wd2": wd2T, "bd1": bd1c, "bm": bmc,
            "enc0": np.ascontiguousarray(enc0T[:, g * BH:(g + 1) * BH]).astype(bf),
        }
        for l in range(L):
            m[f"wih{l}"] = np.ascontiguousarray(W_ih[l][rows, :].T).astype(bf)
            m[f"whh{l}"] = np.ascontiguousarray(W_hh[l][rows, :].T).astype(bf)
            bsum = (b_ih[l] + b_hh[l])[rows]
            m[f"bg{l}"] = np.ascontiguousarray(bsum[None, :]).astype(bf)
        in_maps.append(m)
    return in_maps, bd2


def kernel(x, We, be, W_ih, W_hh, b_ih, b_hh, Wd1, bd1, Wd2, bd2, timesteps, **run_kw):
    tsteps = int(timesteps)
    nc = _get_program(tsteps)
    in_maps, bd2_np = _prep_inputs(x, We, be, W_ih, W_hh, b_ih, b_hh, Wd1, bd1, Wd2, bd2)
    res = run_bass_kernel_spmd(nc, in_maps, core_ids=list(range(NCORES)), **run_kw)
    kernel.last_results = res
    halves = [np.asarray(res.results[g * GP]["out"], np.float32) for g in range(NG)]
    out = np.concatenate(halves, axis=0) + bd2_np[None, None, :]
    return out
